# revision 38
# baseline (speedup 1.0000x reference)
"""2-layer GCN (GridGNN) on 8 Trainium2 NeuronCores.

2D sharding: core c=(q,h), q=c//2 source-quarter (25088 nodes), h=c%2
destination parity group. Core c handles edges with src in quarter q and
dst in shards {s: s%2==h}. Each core ships only its OWN shard of x (fp8);
the per-quarter message table is built on-device by transforming the own
shard and AllGathering within quarter pairs, then cast-DMA'd to a flat
f32 table in HBM (with a trailing zero row for padding). Messages are
moved per 14-window chunk with gpsimd dma_gather (node-id indices) and
accumulated into the f32 partial-aggregate buffer with dma_scatter_add
(SDMA CCE in-order += handles duplicate destinations); partials are
ReduceScattered within parity groups; pooled sums AllReduced;
linear+softmax head on device.
"""
import numpy as np
import ml_dtypes

N_NODES = 100000
N_GRAPHS = 64
F = 64
N_ACT = 3
P = 128
SHARD = 12544
NW = 98
QUART = 2 * SHARD
ZROW = QUART          # zero row appended to the message table
NWIN = 4 * NW
CHUNK_W = 14
HALF = 2 * SHARD      # rows per scatter half-region of rs_in
TCALL = 6272          # max tokens per gather/scatter call
MCOLS = 456           # packed meta tensor columns

bf16 = ml_dtypes.bfloat16
f8e4 = ml_dtypes.float8_e4m3


def _prep(x, edge_index, batch, W1, b1, W2, b2, Wl, bl):
    src = edge_index[0].astype(np.int64)
    dst = edge_index[1].astype(np.int64)
    q_e = src // QUART
    shard_e = dst // SHARD
    core_e = q_e * 2 + (shard_e % 2)

    # Per core: split edges by dst half (2 shard-slots each), rank each edge
    # by its occurrence number within its destination row so that every
    # (half, rank) slice has unique rows -> dma_scatter_add is exact.
    per_core = []          # (gi, rowh, half, rank) arrays, edges sorted
    cnt_hr = {}            # (c, half) -> array of per-rank counts
    trash = np.zeros((8, 2), np.int64)
    for c in range(8):
        m = core_e == c
        s, d = src[m], dst[m]
        sh = d // SHARD
        slot = sh // 2                     # 0..3 within parity group
        dlocal = d - sh * SHARD
        row = slot * SHARD + dlocal        # row in rs_in [4*SHARD]
        half = slot // 2
        rowh = row - half * HALF           # row within half [0, HALF)
        gi = s - (c // 2) * QUART
        # occurrence rank of each edge within (half, rowh)
        key = half * HALF + rowh
        order = np.argsort(key, kind="stable")
        ks = key[order]
        starts = np.r_[0, np.nonzero(np.diff(ks))[0] + 1]
        reps = np.diff(np.r_[starts, ks.size])
        rank_sorted = np.arange(ks.size) - np.repeat(starts, reps)
        rank = np.empty(ks.size, np.int64)
        rank[order] = rank_sorted
        per_core.append((gi, rowh, half, rank))
        for hf in range(2):
            mh = half == hf
            cnt_hr[(c, hf)] = np.bincount(rank[mh]) if mh.any() else \
                np.zeros(1, np.int64)
            # a row with no edges at all in this half (pad target)
            used = np.zeros(HALF, bool)
            used[rowh[mh]] = True
            free = np.nonzero(~used)[0]
            assert free.size > 0, "no zero-degree row in half"
            trash[c, hf] = free[0]

    # call schedule: identical across cores. For each (half, rank, piece):
    # size = 128-aligned max-over-cores piece count, capped at TCALL.
    calls = []                             # (half, rank, size, piece)
    for hf in range(2):
        rmax = max(len(cnt_hr[(c, hf)]) for c in range(8))
        for r in range(rmax):
            mx = max(int(cnt_hr[(c, hf)][r]) if r < len(cnt_hr[(c, hf)])
                     else 0 for c in range(8))
            left, j = mx, 0
            while left > 0:
                sz = -(-min(TCALL, left) // P) * P
                calls.append((hf, r, sz, j))
                left -= TCALL
                j += 1

    Etot = sum(sz for (_, _, sz, _) in calls)
    offs = np.concatenate([[0], np.cumsum([sz for (_, _, sz, _) in calls])])
    chunks = [(calls[i][0], int(offs[i]), int(offs[i + 1]))
              for i in range(len(calls))]   # (half, a, b)

    gkeys = np.array([hf * 4096 + r for (hf, r, _, _) in calls])
    gidx_all = np.full((8, Etot), ZROW, np.int16)
    sidx_all = np.zeros((8, Etot), np.int16)
    for c in range(8):
        gi, rowh, half, rank = per_core[c]
        # sort edges by (half, rank, rowh) for deterministic packing
        gkey = half * 4096 + rank
        order = np.argsort(gkey * np.int64(HALF) + rowh, kind="stable")
        gi, rowh, gkey = gi[order], rowh[order], gkey[order]
        g0 = np.searchsorted(gkey, gkeys, side="left")
        g1 = np.searchsorted(gkey, gkeys, side="right")
        for i, (hf, r, sz, j) in enumerate(calls):
            a = int(offs[i])
            sidx_all[c, a:a + sz] = trash[c, hf]
            s0 = g0[i] + j * TCALL
            n = min(int(g1[i]) - s0, sz)
            if n > 0:
                gidx_all[c, a:a + n] = gi[s0:s0 + n]
                sidx_all[c, a:a + n] = rowh[s0:s0 + n]

    # wrap in 16 partitions (token t at [t%16, t//16]), per call
    def wrap16(v_all):
        out = []
        for c in range(8):
            cols = [v_all[c, a:b].reshape(-1, 16).T for (_, a, b) in chunks]
            out.append(np.concatenate(cols, axis=1))
        return np.stack(out)            # [8, 16, Etot//16]
    gidx_sb = wrap16(gidx_all)
    sidx_sb = wrap16(sidx_all)

    deg = np.zeros(8 * SHARD, np.int64)
    np.add.at(deg, dst, 1)
    xpad = np.zeros((8 * SHARD, F), np.float32)
    xpad[:N_NODES] = x
    bpad = np.full(8 * SHARD, 127, np.float32)
    bpad[:N_NODES] = batch

    in_maps = []
    for c in range(8):
        os_ = slice(c * SHARD, (c + 1) * SHARD)
        meta = np.zeros((P, MCOLS), np.float32)
        meta[:, 0:NW] = deg[os_].reshape(NW, P).T
        meta[:, NW:2 * NW] = bpad[os_].reshape(NW, P).T
        meta[:, 196:260] = np.broadcast_to(b1, (P, F))
        meta[:, 260:324] = np.broadcast_to(b2, (P, F))
        meta[:F, 324:388] = W1
        meta[:, 388:452] = np.concatenate([W2, W2], axis=0)
        meta[:F + 1, 452:456] = _wl_aug(Wl, bl)
        in_maps.append({
            "xo_T": np.ascontiguousarray(xpad[os_].T.astype(f8e4)),
            "idx": np.ascontiguousarray(
                np.concatenate([gidx_sb[c], sidx_sb[c]], axis=1)),
            "meta": meta.astype(bf16),
        })
    return in_maps, calls, chunks


def _wl_aug(Wl, bl):
    Wl_aug = np.zeros((F + 1, 4), np.float32)
    Wl_aug[:F, :3] = Wl
    Wl_aug[F, :3] = bl
    Wl_aug[F, 3] = 1.0
    return Wl_aug


def _build(calls, chunks):
    import concourse.bass as bass
    import concourse.bacc as bacc
    import concourse.tile as tile
    import concourse.mybir as mybir
    from concourse.library_config import mlp
    from concourse.masks import make_identity

    Etot = chunks[-1][2]
    nc = bacc.Bacc("TRN2", target_bir_lowering=False, debug=False,
                   num_devices=8)
    F32, BF, I16 = mybir.dt.float32, mybir.dt.bfloat16, mybir.dt.int16
    F8 = mybir.dt.float8e4
    AF = mybir.ActivationFunctionType
    OP = mybir.AluOpType

    def ein(name, shape, dt):
        return nc.dram_tensor(name, shape, dt, kind="ExternalInput")

    xo_T = ein("xo_T", [F, SHARD], F8)
    idxh = ein("idx", [16, 2 * (Etot // 16)], I16)
    metah = ein("meta", [P, MCOLS], BF)
    out_h = nc.dram_tensor("out", [N_GRAPHS, N_ACT], F32,
                           kind="ExternalOutput")

    ftab = [nc.dram_tensor(f"ftab{i}", [QUART + P, F], F32, kind="Internal")
            for i in range(2)]
    rs_in = [nc.dram_tensor(f"rs_in{i}", [4 * SHARD, F], F32, kind="Internal")
             for i in range(2)]
    rs_out = [nc.dram_tensor(f"rs_out{i}", [SHARD, F], F32, kind="Internal")
              for i in range(2)]
    ag_in = [nc.dram_tensor(f"ag_in{i}", [SHARD, F], BF, kind="Internal")
             for i in range(2)]
    ag_out = [nc.dram_tensor(f"ag_out{i}", [QUART, F], BF, kind="Internal")
              for i in range(2)]
    pool_in = nc.dram_tensor("pool_in", [F + 1, N_GRAPHS], F32,
                             kind="Internal")
    pool_out = nc.dram_tensor("pool_out", [F + 1, N_GRAPHS], F32,
                              kind="Internal", addr_space="Shared")

    RG2 = [[0, 1], [2, 3], [4, 5], [6, 7]]
    RGH = [[0, 2, 4, 6], [1, 3, 5, 7]]
    RG8 = [[0, 1, 2, 3, 4, 5, 6, 7]]

    nc.gpsimd.load_library(mlp)
    with tile.TileContext(nc) as tc:
        with tc.tile_pool(name="cst", bufs=1) as cst, \
             tc.tile_pool(name="big", bufs=1) as big, \
             tc.tile_pool(name="mv", bufs=2) as mv, \
             tc.tile_pool(name="ps", bufs=2, space="PSUM") as ps, \
             tc.tile_pool(name="pw", bufs=2, space="PSUM") as pw, \
             tc.tile_pool(name="pc", bufs=1, space="PSUM") as pc:

            ident = cst.tile([P, P], BF)
            make_identity(nc, ident[:])
            iota_i = cst.tile([P, N_GRAPHS], mybir.dt.int32)
            nc.gpsimd.iota(iota_i[:], pattern=[[1, N_GRAPHS]], base=0,
                           channel_multiplier=0)
            iota = cst.tile([P, N_GRAPHS], BF)
            nc.vector.tensor_copy(out=iota[:], in_=iota_i[:])

            metat = cst.tile([P, MCOLS], BF)
            nc.sync.dma_start(out=metat[:], in_=metah.ap())
            batt = metat[:, NW:2 * NW]
            b1t = metat[:, 196:260]
            b2t = metat[:, 260:324]
            W1t = metat[:F, 324:388]
            W2t = metat[:, 388:452]
            # replicate compact idx lists across the 8 channel groups
            idxg = cst.tile([P, Etot // 16], I16)
            idxs = cst.tile([P, Etot // 16], I16)
            ecols = Etot // 16
            for k in range(8):
                nc.sync.dma_start(out=idxg[16 * k:16 * (k + 1), :],
                                  in_=idxh.ap()[:, :ecols])
                nc.sync.dma_start(out=idxs[16 * k:16 * (k + 1), :],
                                  in_=idxh.ap()[:, ecols:])

            zC = cst.tile([P, CHUNK_W * F], F32)
            nc.vector.memset(zC[:], 0.0)
            # zero rows ZROW..ZROW+P of both message tables (padding target)
            for li in range(2):
                nc.sync.dma_start(out=ftab[li].ap()[ZROW:ZROW + P, :],
                                  in_=zC[:, :F])

            dinvo = cst.tile([P, NW], F32)
            nc.vector.tensor_copy(out=dinvo[:], in_=metat[:, :NW])
            nc.vector.tensor_scalar(out=dinvo[:], in0=dinvo[:], scalar1=1.0,
                                    scalar2=None, op0=OP.add)
            nc.vector.reciprocal(out=dinvo[:], in_=dinvo[:])
            nc.scalar.activation(dinvo[:], dinvo[:], AF.Sqrt)
            dvb = dinvo[:].unsqueeze(2).to_broadcast([P, NW, F])

            tso = big.tile([P, NW * F], BF)      # (x@W1)*dinv, own shard
            h1own = big.tile([P, NW * F], BF)
            self2 = big.tile([P, NW * F], BF)
            ts2all = big.tile([P, NW * F], BF)
            h2aug = big.tile([P, NW * (F + 1)], BF)
            agg = big.tile([P, NW * F], BF)

            tso3 = tso[:].rearrange("p (t f) -> p t f", f=F)

            # ---- layer 1 transform (own shard), streamed ----
            XC = 14
            for t0 in range(0, NW, XC):
                t1 = min(t0 + XC, NW)
                xc8 = mv.tile([F, XC * P], F8, tag="xc8")
                nc.sync.dma_start(out=xc8[:, :(t1 - t0) * P],
                                  in_=xo_T.ap()[:, t0 * P:t1 * P])
                xc = mv.tile([F, XC * P], BF, tag="xc")
                nc.vector.tensor_copy(out=xc[:, :(t1 - t0) * P],
                                      in_=xc8[:, :(t1 - t0) * P])
                for t in range(t0, t1):
                    pt = pw.tile([P, F], F32, space="PSUM", tag="tr")
                    nc.tensor.matmul(
                        out=pt[:], lhsT=xc[:, (t - t0) * P:(t - t0 + 1) * P],
                        rhs=W1t, start=True, stop=True)
                    nc.vector.tensor_tensor(
                        out=tso3[:, t, :], in0=pt[:],
                        in1=dinvo[:, t:t + 1].to_broadcast([P, F]),
                        op=OP.mult)
            nc.sync.dma_start(
                out=ag_in[0].ap().rearrange("(w p) f -> p w f", p=P),
                in_=tso3)
            nc.gpsimd.collective_compute(
                "AllGather", OP.bypass, replica_groups=RG2,
                ins=[ag_in[0].ap()], outs=[ag_out[0].ap()])
            nc.gpsimd.dma_start(out=ftab[0].ap()[:QUART, :],
                                in_=ag_out[0].ap())

            MSZ = TCALL // P

            def edge_phase(li):
                for w0 in range(0, NWIN, CHUNK_W):
                    nc.sync.dma_start(
                        out=rs_in[li].ap()[w0 * P:(w0 + CHUNK_W) * P, :]
                            .rearrange("(w p) f -> p w f", p=P),
                        in_=zC[:].rearrange("p (w f) -> p w f", f=F))
                for (hf, a, b) in chunks:
                    nt = (b - a) // P
                    msg = mv.tile([P, MSZ * F], F32, tag="msg")
                    nc.gpsimd.dma_gather(
                        out_ap=msg[:, :nt * F].rearrange(
                            "p (t f) -> p t f", f=F),
                        in_ap=ftab[li].ap(),
                        idxs_ap=idxg[:, a // 16:b // 16],
                        num_idxs=b - a,
                        num_idxs_reg=b - a,
                        elem_size=F,
                        single_packet=False,
                    )
                    nc.gpsimd.dma_scatter_add(
                        out_ap=rs_in[li].ap()[hf * HALF:(hf + 1) * HALF, :],
                        in_ap=msg[:, :nt * F].rearrange(
                            "p (t f) -> p t f", f=F),
                        idxs_ap=idxs[:, a // 16:b // 16],
                        num_idxs=b - a,
                        num_idxs_reg=b - a,
                        elem_size=F,
                    )
                nc.gpsimd.collective_compute(
                    "ReduceScatter", OP.add, replica_groups=RGH,
                    ins=[rs_in[li].ap()], outs=[rs_out[li].ap()])

            def load_agg(li):
                a3 = agg[:].rearrange("p (w f) -> p w f", f=F)
                for w0 in range(0, NW, CHUNK_W):
                    w1 = min(w0 + CHUNK_W, NW)
                    ar = mv.tile([P, CHUNK_W * F], F32, tag="ar")
                    nc.sync.dma_start(
                        out=ar[:, :(w1 - w0) * F].rearrange(
                            "p (w f) -> p w f", f=F),
                        in_=rs_out[li].ap()[w0 * P:w1 * P, :].rearrange(
                            "(w p) f -> p w f", p=P))
                    nc.vector.tensor_copy(
                        out=a3[:, w0:w1, :],
                        in_=ar[:, :(w1 - w0) * F].rearrange(
                            "p (w f) -> p w f", f=F))
                return a3

            # ---- layer 1 ----
            edge_phase(0)
            a3 = load_agg(0)
            h3 = h1own[:].rearrange("p (w f) -> p w f", f=F)
            # h1 = relu((agg + tso) * dinv + b1)
            nc.vector.tensor_tensor(out=h3[:], in0=a3[:], in1=tso3[:],
                                    op=OP.add)
            nc.vector.tensor_tensor(out=h3[:], in0=h3[:], in1=dvb,
                                    op=OP.mult)
            nc.vector.tensor_tensor(
                out=h3[:], in0=h3[:],
                in1=b1t.unsqueeze(1).to_broadcast([P, NW, F]), op=OP.add)
            nc.vector.tensor_scalar(out=h1own[:], in0=h1own[:],
                                    scalar1=0.0, scalar2=None, op0=OP.max)

            # ---- layer 2 transform (own shard): pairs of windows ----
            t23 = ts2all[:].rearrange("p (w f) -> p w f", f=F)
            for wp in range(0, NW, 2):
                trp = pc.tile([P, P], BF, space="PSUM", tag="trp")
                nc.tensor.transpose(out=trp[:],
                                    in_=h1own[:, wp * F:(wp + 2) * F],
                                    identity=ident[:])
                h1T = mv.tile([P, P], BF, tag="h1T")
                nc.vector.tensor_copy(out=h1T[:], in_=trp[:])
                for j in range(2):
                    w = wp + j
                    pt = pw.tile([P, F], F32, space="PSUM", tag="tr")
                    nc.tensor.matmul(out=pt[:], lhsT=h1T[j * F:(j + 1) * F, :],
                                     rhs=metat[j * F:(j + 1) * F, 388:452],
                                     start=True, stop=True)
                    nc.vector.tensor_tensor(
                        out=t23[:, w, :], in0=pt[:],
                        in1=dinvo[:, w:w + 1].to_broadcast([P, F]),
                        op=OP.mult)
            s23 = self2[:].rearrange("p (w f) -> p w f", f=F)
            nc.vector.tensor_tensor(out=s23[:], in0=t23[:], in1=dvb,
                                    op=OP.mult)
            nc.sync.dma_start(
                out=ag_in[1].ap().rearrange("(w p) f -> p w f", p=P),
                in_=t23)
            nc.gpsimd.collective_compute(
                "AllGather", OP.bypass, replica_groups=RG2,
                ins=[ag_in[1].ap()], outs=[ag_out[1].ap()])
            nc.gpsimd.dma_start(out=ftab[1].ap()[:QUART, :],
                                in_=ag_out[1].ap())

            # ---- layer 2 ----
            edge_phase(1)
            a23 = load_agg(1)
            h2a3 = h2aug[:].rearrange("p (w g) -> p w g", g=F + 1)
            nc.vector.memset(h2aug[:], 1.0)
            h2f = h2a3[:, :, :F]
            nc.vector.tensor_tensor(out=h2f, in0=a23[:], in1=dvb, op=OP.mult)
            nc.vector.tensor_tensor(out=h2f, in0=h2f, in1=s23[:], op=OP.add)
            nc.vector.tensor_tensor(
                out=h2f, in0=h2f,
                in1=b2t.unsqueeze(1).to_broadcast([P, NW, F]), op=OP.add)

            # ---- pooling ----
            ohg = big.tile([P, NW * N_GRAPHS], BF)
            nc.vector.tensor_tensor(
                out=ohg[:].rearrange("p (w g) -> p w g", g=N_GRAPHS),
                in0=batt.unsqueeze(2).to_broadcast([P, NW, N_GRAPHS]),
                in1=iota[:].unsqueeze(1).to_broadcast([P, NW, N_GRAPHS]),
                op=OP.is_equal)
            poolp = pc.tile([F + 1, N_GRAPHS], F32, space="PSUM", tag="pool")
            for w in range(NW):
                nc.tensor.matmul(out=poolp[:], lhsT=h2a3[:, w, :],
                                 rhs=ohg[:, w * N_GRAPHS:(w + 1) * N_GRAPHS],
                                 start=(w == 0), stop=(w == NW - 1))
            pools = cst.tile([F + 1, N_GRAPHS], F32)
            nc.vector.tensor_copy(out=pools[:], in_=poolp[:])
            nc.sync.dma_start(out=pool_in.ap(), in_=pools[:])
            nc.gpsimd.collective_compute(
                "AllReduce", OP.add, replica_groups=RG8,
                ins=[pool_in.ap()], outs=[pool_out.ap()])

            # ---- head ----
            pooled = cst.tile([F + 1, N_GRAPHS], F32)
            nc.sync.dma_start(out=pooled[:], in_=pool_out.ap())
            poolb = cst.tile([F + 1, N_GRAPHS], BF)
            nc.vector.tensor_copy(out=poolb[:], in_=pooled[:])
            zp = pc.tile([4, N_GRAPHS], F32, space="PSUM", tag="z")
            nc.tensor.matmul(out=zp[:], lhsT=metat[:F + 1, 452:456],
                             rhs=poolb[:], start=True, stop=True)
            zs = cst.tile([4, N_GRAPHS], F32)
            nc.vector.tensor_copy(out=zs[:], in_=zp[:])
            identf = cst.tile([P, P], F32)
            make_identity(nc, identf[:])
            ztp = pc.tile([N_GRAPHS, 4], F32, space="PSUM", tag="zt")
            nc.tensor.transpose(out=ztp[:], in_=zs[:], identity=identf[:4, :4])
            zt = cst.tile([N_GRAPHS, 4], F32)
            nc.vector.tensor_copy(out=zt[:], in_=ztp[:])
            rc = cst.tile([N_GRAPHS, 1], F32)
            nc.vector.reciprocal(out=rc[:], in_=zt[:, 3:4])
            lg = cst.tile([N_GRAPHS, N_ACT], F32)
            nc.vector.tensor_tensor(out=lg[:], in0=zt[:, :N_ACT],
                                    in1=rc[:].to_broadcast([N_GRAPHS, N_ACT]),
                                    op=OP.mult)
            mx = cst.tile([N_GRAPHS, 1], F32)
            nc.vector.tensor_reduce(out=mx[:], in_=lg[:], op=OP.max,
                                    axis=mybir.AxisListType.X)
            nc.vector.tensor_tensor(
                out=lg[:], in0=lg[:],
                in1=mx[:].to_broadcast([N_GRAPHS, N_ACT]), op=OP.subtract)
            nc.scalar.activation(lg[:], lg[:], AF.Exp)
            sm = cst.tile([N_GRAPHS, 1], F32)
            nc.vector.tensor_reduce(out=sm[:], in_=lg[:], op=OP.add,
                                    axis=mybir.AxisListType.X)
            nc.vector.reciprocal(out=sm[:], in_=sm[:])
            nc.vector.tensor_tensor(
                out=lg[:], in0=lg[:],
                in1=sm[:].to_broadcast([N_GRAPHS, N_ACT]), op=OP.mult)
            nc.sync.dma_start(out=out_h.ap(), in_=lg[:])

    nc.compile()
    return nc


def kernel(x, edge_index, batch, W1, b1, W2, b2, Wl, bl):
    from concourse.bass_utils import run_bass_kernel_spmd
    in_maps, calls, chunks = _prep(np.asarray(x), np.asarray(edge_index),
                                   np.asarray(batch), np.asarray(W1),
                                   np.asarray(b1), np.asarray(W2),
                                   np.asarray(b2), np.asarray(Wl),
                                   np.asarray(bl))
    nc = _build(calls, chunks)
    res = run_bass_kernel_spmd(nc, in_maps, core_ids=list(range(8)))
    return np.asarray(res.results[0]["out"], dtype=np.float32)


# revision 40
# speedup vs baseline: 2.7615x; 2.7615x over previous
"""2-layer GCN (GridGNN) on 8 Trainium2 NeuronCores.

2D sharding: core c=(q,h), q=c//2 source-quarter (25088 nodes), h=c%2
destination parity group. Core c handles edges with src in quarter q and
dst in shards {s: s%2==h}. Each core ships only its OWN shard of x (fp8,
~0.8MB); the per-quarter message table is built on-device by transforming
the own shard and AllGathering within quarter pairs, then cast-DMA'd to a
flat f32 table in HBM (trailing zero row as gather-padding target).
Messages move via gpsimd dma_gather (source-node indices) and accumulate
into the f32 partial-aggregate HBM buffer via dma_scatter_add. The SDMA
CCE += loses updates when a call contains duplicate destination rows
(verified on HW), so edges are ranked host-side by occurrence number
within their destination row and emitted as one gather+scatter call per
(dst-half, rank, piece) — rows within a call are then unique, and calls
targeting the same half are serialized by the tile framework's WAW
semaphores. Padding tokens gather the zero row and scatter onto a
zero-degree row, so they cannot race real updates. Partials are
ReduceScattered within parity groups; pooled sums (count-augmented via a
homogeneous column) are AllReduced; linear+softmax head on device. All
small per-core constants ship as one packed bf16 tensor to minimize
per-array transfer round-trips over the axon tunnel.
"""
import numpy as np
import ml_dtypes

N_NODES = 100000
N_GRAPHS = 64
F = 64
N_ACT = 3
P = 128
SHARD = 12544
NW = 98
QUART = 2 * SHARD
ZROW = QUART          # zero row appended to the message table
NWIN = 4 * NW
CHUNK_W = 14
HALF = 2 * SHARD      # rows per scatter half-region of rs_in
TCALL = 6272          # max tokens per gather/scatter call
MCOLS = 456           # packed meta tensor columns
RMUL = 1 << 20        # (half, rank) sort-key multiplier

bf16 = ml_dtypes.bfloat16
f8e4 = ml_dtypes.float8_e4m3


def _prep(x, edge_index, batch, W1, b1, W2, b2, Wl, bl):
    src = edge_index[0].astype(np.int64)
    dst = edge_index[1].astype(np.int64)
    q_e = src // QUART
    shard_e = dst // SHARD
    core_e = q_e * 2 + (shard_e % 2)

    # Per core: split edges by dst half (2 shard-slots each), rank each edge
    # by its occurrence number within its destination row so that every
    # (half, rank) slice has unique rows -> dma_scatter_add is exact.
    per_core = []          # (gi, rowh, half, rank) arrays, edges sorted
    cnt_hr = {}            # (c, half) -> array of per-rank counts
    trash = np.zeros((8, 2), np.int64)
    for c in range(8):
        m = core_e == c
        s, d = src[m], dst[m]
        sh = d // SHARD
        slot = sh // 2                     # 0..3 within parity group
        dlocal = d - sh * SHARD
        row = slot * SHARD + dlocal        # row in rs_in [4*SHARD]
        half = slot // 2
        rowh = row - half * HALF           # row within half [0, HALF)
        gi = s - (c // 2) * QUART
        # occurrence rank of each edge within (half, rowh)
        key = half * HALF + rowh
        order = np.argsort(key, kind="stable")
        ks = key[order]
        starts = np.r_[0, np.nonzero(np.diff(ks))[0] + 1]
        reps = np.diff(np.r_[starts, ks.size])
        rank_sorted = np.arange(ks.size) - np.repeat(starts, reps)
        rank = np.empty(ks.size, np.int64)
        rank[order] = rank_sorted
        per_core.append((gi, rowh, half, rank))
        for hf in range(2):
            mh = half == hf
            cnt_hr[(c, hf)] = np.bincount(rank[mh]) if mh.any() else \
                np.zeros(1, np.int64)
            # a row with no edges at all in this half (pad target)
            used = np.zeros(HALF, bool)
            used[rowh[mh]] = True
            free = np.nonzero(~used)[0]
            assert free.size > 0, "no zero-degree row in half"
            trash[c, hf] = free[0]

    # call schedule: identical across cores. For each (half, rank, piece):
    # size = 128-aligned max-over-cores piece count, capped at TCALL.
    calls = []                             # (half, rank, size, piece)
    for hf in range(2):
        rmax = max(len(cnt_hr[(c, hf)]) for c in range(8))
        for r in range(rmax):
            mx = max(int(cnt_hr[(c, hf)][r]) if r < len(cnt_hr[(c, hf)])
                     else 0 for c in range(8))
            left, j = mx, 0
            while left > 0:
                sz = -(-min(TCALL, left) // P) * P
                calls.append((hf, r, sz, j))
                left -= TCALL
                j += 1

    Etot = sum(sz for (_, _, sz, _) in calls)
    offs = np.concatenate([[0], np.cumsum([sz for (_, _, sz, _) in calls])])
    chunks = [(calls[i][0], int(offs[i]), int(offs[i + 1]))
              for i in range(len(calls))]   # (half, a, b)

    gkeys = np.array([hf * RMUL + r for (hf, r, _, _) in calls])
    gidx_all = np.full((8, Etot), ZROW, np.int16)
    sidx_all = np.zeros((8, Etot), np.int16)
    for c in range(8):
        gi, rowh, half, rank = per_core[c]
        # sort edges by (half, rank, rowh) for deterministic packing
        gkey = half * RMUL + rank
        order = np.argsort(gkey * np.int64(HALF) + rowh, kind="stable")
        gi, rowh, gkey = gi[order], rowh[order], gkey[order]
        g0 = np.searchsorted(gkey, gkeys, side="left")
        g1 = np.searchsorted(gkey, gkeys, side="right")
        for i, (hf, r, sz, j) in enumerate(calls):
            a = int(offs[i])
            sidx_all[c, a:a + sz] = trash[c, hf]
            s0 = g0[i] + j * TCALL
            n = min(int(g1[i]) - s0, sz)
            if n > 0:
                gidx_all[c, a:a + n] = gi[s0:s0 + n]
                sidx_all[c, a:a + n] = rowh[s0:s0 + n]

    # wrap in 16 partitions (token t at [t%16, t//16]), per call
    def wrap16(v_all):
        out = []
        for c in range(8):
            cols = [v_all[c, a:b].reshape(-1, 16).T for (_, a, b) in chunks]
            out.append(np.concatenate(cols, axis=1))
        return np.stack(out)            # [8, 16, Etot//16]
    gidx_sb = wrap16(gidx_all)
    sidx_sb = wrap16(sidx_all)

    deg = np.zeros(8 * SHARD, np.int64)
    np.add.at(deg, dst, 1)
    xpad = np.zeros((8 * SHARD, F), np.float32)
    xpad[:N_NODES] = x
    bpad = np.full(8 * SHARD, 127, np.float32)
    bpad[:N_NODES] = batch

    in_maps = []
    for c in range(8):
        os_ = slice(c * SHARD, (c + 1) * SHARD)
        meta = np.zeros((P, MCOLS), np.float32)
        meta[:, 0:NW] = deg[os_].reshape(NW, P).T
        meta[:, NW:2 * NW] = bpad[os_].reshape(NW, P).T
        meta[:, 196:260] = np.broadcast_to(b1, (P, F))
        meta[:, 260:324] = np.broadcast_to(b2, (P, F))
        meta[:F, 324:388] = W1
        meta[:, 388:452] = np.concatenate([W2, W2], axis=0)
        meta[:F + 1, 452:456] = _wl_aug(Wl, bl)
        in_maps.append({
            "xo_T": np.ascontiguousarray(xpad[os_].T.astype(f8e4)),
            "idx": np.ascontiguousarray(
                np.concatenate([gidx_sb[c], sidx_sb[c]], axis=1)),
            "meta": meta.astype(bf16),
        })
    return in_maps, calls, chunks


def _wl_aug(Wl, bl):
    Wl_aug = np.zeros((F + 1, 4), np.float32)
    Wl_aug[:F, :3] = Wl
    Wl_aug[F, :3] = bl
    Wl_aug[F, 3] = 1.0
    return Wl_aug


def _build(calls, chunks):
    import concourse.bass as bass
    import concourse.bacc as bacc
    import concourse.tile as tile
    import concourse.mybir as mybir
    from concourse.library_config import mlp
    from concourse.masks import make_identity

    Etot = chunks[-1][2]
    nc = bacc.Bacc("TRN2", target_bir_lowering=False, debug=False,
                   num_devices=8)
    F32, BF, I16 = mybir.dt.float32, mybir.dt.bfloat16, mybir.dt.int16
    F8 = mybir.dt.float8e4
    AF = mybir.ActivationFunctionType
    OP = mybir.AluOpType

    def ein(name, shape, dt):
        return nc.dram_tensor(name, shape, dt, kind="ExternalInput")

    xo_T = ein("xo_T", [F, SHARD], F8)
    idxh = ein("idx", [16, 2 * (Etot // 16)], I16)
    metah = ein("meta", [P, MCOLS], BF)
    out_h = nc.dram_tensor("out", [N_GRAPHS, N_ACT], F32,
                           kind="ExternalOutput")

    ftab = [nc.dram_tensor(f"ftab{i}", [QUART + P, F], F32, kind="Internal")
            for i in range(2)]
    rs_in = [nc.dram_tensor(f"rs_in{i}", [4 * SHARD, F], F32, kind="Internal")
             for i in range(2)]
    rs_out = [nc.dram_tensor(f"rs_out{i}", [SHARD, F], F32, kind="Internal")
              for i in range(2)]
    ag_in = [nc.dram_tensor(f"ag_in{i}", [SHARD, F], BF, kind="Internal")
             for i in range(2)]
    ag_out = [nc.dram_tensor(f"ag_out{i}", [QUART, F], BF, kind="Internal")
              for i in range(2)]
    pool_in = nc.dram_tensor("pool_in", [F + 1, N_GRAPHS], F32,
                             kind="Internal")
    pool_out = nc.dram_tensor("pool_out", [F + 1, N_GRAPHS], F32,
                              kind="Internal", addr_space="Shared")

    RG2 = [[0, 1], [2, 3], [4, 5], [6, 7]]
    RGH = [[0, 2, 4, 6], [1, 3, 5, 7]]
    RG8 = [[0, 1, 2, 3, 4, 5, 6, 7]]

    nc.gpsimd.load_library(mlp)
    with tile.TileContext(nc) as tc:
        with tc.tile_pool(name="cst", bufs=1) as cst, \
             tc.tile_pool(name="big", bufs=1) as big, \
             tc.tile_pool(name="mv", bufs=2) as mv, \
             tc.tile_pool(name="ps", bufs=2, space="PSUM") as ps, \
             tc.tile_pool(name="pw", bufs=2, space="PSUM") as pw, \
             tc.tile_pool(name="pc", bufs=1, space="PSUM") as pc:

            ident = cst.tile([P, P], BF)
            make_identity(nc, ident[:])
            iota_i = cst.tile([P, N_GRAPHS], mybir.dt.int32)
            nc.gpsimd.iota(iota_i[:], pattern=[[1, N_GRAPHS]], base=0,
                           channel_multiplier=0)
            iota = cst.tile([P, N_GRAPHS], BF)
            nc.vector.tensor_copy(out=iota[:], in_=iota_i[:])

            metat = cst.tile([P, MCOLS], BF)
            nc.sync.dma_start(out=metat[:], in_=metah.ap())
            batt = metat[:, NW:2 * NW]
            b1t = metat[:, 196:260]
            b2t = metat[:, 260:324]
            W1t = metat[:F, 324:388]
            W2t = metat[:, 388:452]
            # replicate compact idx lists across the 8 channel groups
            idxg = cst.tile([P, Etot // 16], I16)
            idxs = cst.tile([P, Etot // 16], I16)
            ecols = Etot // 16
            for k in range(8):
                nc.sync.dma_start(out=idxg[16 * k:16 * (k + 1), :],
                                  in_=idxh.ap()[:, :ecols])
                nc.sync.dma_start(out=idxs[16 * k:16 * (k + 1), :],
                                  in_=idxh.ap()[:, ecols:])

            zC = cst.tile([P, CHUNK_W * F], F32)
            nc.vector.memset(zC[:], 0.0)
            # zero rows ZROW..ZROW+P of both message tables (padding target)
            for li in range(2):
                nc.sync.dma_start(out=ftab[li].ap()[ZROW:ZROW + P, :],
                                  in_=zC[:, :F])

            dinvo = cst.tile([P, NW], F32)
            nc.vector.tensor_copy(out=dinvo[:], in_=metat[:, :NW])
            nc.vector.tensor_scalar(out=dinvo[:], in0=dinvo[:], scalar1=1.0,
                                    scalar2=None, op0=OP.add)
            nc.vector.reciprocal(out=dinvo[:], in_=dinvo[:])
            nc.scalar.activation(dinvo[:], dinvo[:], AF.Sqrt)
            dvb = dinvo[:].unsqueeze(2).to_broadcast([P, NW, F])

            tso = big.tile([P, NW * F], BF)      # (x@W1)*dinv, own shard
            h1own = big.tile([P, NW * F], BF)
            self2 = big.tile([P, NW * F], BF)
            ts2all = big.tile([P, NW * F], BF)
            h2aug = big.tile([P, NW * (F + 1)], BF)
            agg = big.tile([P, NW * F], BF)

            tso3 = tso[:].rearrange("p (t f) -> p t f", f=F)

            # ---- layer 1 transform (own shard), streamed ----
            XC = 14
            for t0 in range(0, NW, XC):
                t1 = min(t0 + XC, NW)
                xc8 = mv.tile([F, XC * P], F8, tag="xc8")
                nc.sync.dma_start(out=xc8[:, :(t1 - t0) * P],
                                  in_=xo_T.ap()[:, t0 * P:t1 * P])
                xc = mv.tile([F, XC * P], BF, tag="xc")
                nc.vector.tensor_copy(out=xc[:, :(t1 - t0) * P],
                                      in_=xc8[:, :(t1 - t0) * P])
                for t in range(t0, t1):
                    pt = pw.tile([P, F], F32, space="PSUM", tag="tr")
                    nc.tensor.matmul(
                        out=pt[:], lhsT=xc[:, (t - t0) * P:(t - t0 + 1) * P],
                        rhs=W1t, start=True, stop=True)
                    nc.vector.tensor_tensor(
                        out=tso3[:, t, :], in0=pt[:],
                        in1=dinvo[:, t:t + 1].to_broadcast([P, F]),
                        op=OP.mult)
            nc.sync.dma_start(
                out=ag_in[0].ap().rearrange("(w p) f -> p w f", p=P),
                in_=tso3)
            nc.gpsimd.collective_compute(
                "AllGather", OP.bypass, replica_groups=RG2,
                ins=[ag_in[0].ap()], outs=[ag_out[0].ap()])
            nc.gpsimd.dma_start(out=ftab[0].ap()[:QUART, :],
                                in_=ag_out[0].ap())

            MSZ = TCALL // P

            def edge_phase(li):
                for w0 in range(0, NWIN, CHUNK_W):
                    nc.sync.dma_start(
                        out=rs_in[li].ap()[w0 * P:(w0 + CHUNK_W) * P, :]
                            .rearrange("(w p) f -> p w f", p=P),
                        in_=zC[:].rearrange("p (w f) -> p w f", f=F))
                for (hf, a, b) in chunks:
                    nt = (b - a) // P
                    msg = mv.tile([P, MSZ * F], F32, tag="msg")
                    nc.gpsimd.dma_gather(
                        out_ap=msg[:, :nt * F].rearrange(
                            "p (t f) -> p t f", f=F),
                        in_ap=ftab[li].ap(),
                        idxs_ap=idxg[:, a // 16:b // 16],
                        num_idxs=b - a,
                        num_idxs_reg=b - a,
                        elem_size=F,
                        single_packet=False,
                    )
                    nc.gpsimd.dma_scatter_add(
                        out_ap=rs_in[li].ap()[hf * HALF:(hf + 1) * HALF, :],
                        in_ap=msg[:, :nt * F].rearrange(
                            "p (t f) -> p t f", f=F),
                        idxs_ap=idxs[:, a // 16:b // 16],
                        num_idxs=b - a,
                        num_idxs_reg=b - a,
                        elem_size=F,
                    )
                nc.gpsimd.collective_compute(
                    "ReduceScatter", OP.add, replica_groups=RGH,
                    ins=[rs_in[li].ap()], outs=[rs_out[li].ap()])

            def load_agg(li):
                a3 = agg[:].rearrange("p (w f) -> p w f", f=F)
                for w0 in range(0, NW, CHUNK_W):
                    w1 = min(w0 + CHUNK_W, NW)
                    ar = mv.tile([P, CHUNK_W * F], F32, tag="ar")
                    nc.sync.dma_start(
                        out=ar[:, :(w1 - w0) * F].rearrange(
                            "p (w f) -> p w f", f=F),
                        in_=rs_out[li].ap()[w0 * P:w1 * P, :].rearrange(
                            "(w p) f -> p w f", p=P))
                    nc.vector.tensor_copy(
                        out=a3[:, w0:w1, :],
                        in_=ar[:, :(w1 - w0) * F].rearrange(
                            "p (w f) -> p w f", f=F))
                return a3

            # ---- layer 1 ----
            edge_phase(0)
            a3 = load_agg(0)
            h3 = h1own[:].rearrange("p (w f) -> p w f", f=F)
            # h1 = relu((agg + tso) * dinv + b1)
            nc.vector.tensor_tensor(out=h3[:], in0=a3[:], in1=tso3[:],
                                    op=OP.add)
            nc.vector.tensor_tensor(out=h3[:], in0=h3[:], in1=dvb,
                                    op=OP.mult)
            nc.vector.tensor_tensor(
                out=h3[:], in0=h3[:],
                in1=b1t.unsqueeze(1).to_broadcast([P, NW, F]), op=OP.add)
            nc.vector.tensor_scalar(out=h1own[:], in0=h1own[:],
                                    scalar1=0.0, scalar2=None, op0=OP.max)

            # ---- layer 2 transform (own shard): pairs of windows ----
            t23 = ts2all[:].rearrange("p (w f) -> p w f", f=F)
            for wp in range(0, NW, 2):
                trp = pc.tile([P, P], BF, space="PSUM", tag="trp")
                nc.tensor.transpose(out=trp[:],
                                    in_=h1own[:, wp * F:(wp + 2) * F],
                                    identity=ident[:])
                h1T = mv.tile([P, P], BF, tag="h1T")
                nc.vector.tensor_copy(out=h1T[:], in_=trp[:])
                for j in range(2):
                    w = wp + j
                    pt = pw.tile([P, F], F32, space="PSUM", tag="tr")
                    nc.tensor.matmul(out=pt[:], lhsT=h1T[j * F:(j + 1) * F, :],
                                     rhs=metat[j * F:(j + 1) * F, 388:452],
                                     start=True, stop=True)
                    nc.vector.tensor_tensor(
                        out=t23[:, w, :], in0=pt[:],
                        in1=dinvo[:, w:w + 1].to_broadcast([P, F]),
                        op=OP.mult)
            s23 = self2[:].rearrange("p (w f) -> p w f", f=F)
            nc.vector.tensor_tensor(out=s23[:], in0=t23[:], in1=dvb,
                                    op=OP.mult)
            nc.sync.dma_start(
                out=ag_in[1].ap().rearrange("(w p) f -> p w f", p=P),
                in_=t23)
            nc.gpsimd.collective_compute(
                "AllGather", OP.bypass, replica_groups=RG2,
                ins=[ag_in[1].ap()], outs=[ag_out[1].ap()])
            nc.gpsimd.dma_start(out=ftab[1].ap()[:QUART, :],
                                in_=ag_out[1].ap())

            # ---- layer 2 ----
            edge_phase(1)
            a23 = load_agg(1)
            h2a3 = h2aug[:].rearrange("p (w g) -> p w g", g=F + 1)
            nc.vector.memset(h2aug[:], 1.0)
            h2f = h2a3[:, :, :F]
            nc.vector.tensor_tensor(out=h2f, in0=a23[:], in1=dvb, op=OP.mult)
            nc.vector.tensor_tensor(out=h2f, in0=h2f, in1=s23[:], op=OP.add)
            nc.vector.tensor_tensor(
                out=h2f, in0=h2f,
                in1=b2t.unsqueeze(1).to_broadcast([P, NW, F]), op=OP.add)

            # ---- pooling ----
            ohg = big.tile([P, NW * N_GRAPHS], BF)
            nc.vector.tensor_tensor(
                out=ohg[:].rearrange("p (w g) -> p w g", g=N_GRAPHS),
                in0=batt.unsqueeze(2).to_broadcast([P, NW, N_GRAPHS]),
                in1=iota[:].unsqueeze(1).to_broadcast([P, NW, N_GRAPHS]),
                op=OP.is_equal)
            poolp = pc.tile([F + 1, N_GRAPHS], F32, space="PSUM", tag="pool")
            for w in range(NW):
                nc.tensor.matmul(out=poolp[:], lhsT=h2a3[:, w, :],
                                 rhs=ohg[:, w * N_GRAPHS:(w + 1) * N_GRAPHS],
                                 start=(w == 0), stop=(w == NW - 1))
            pools = cst.tile([F + 1, N_GRAPHS], F32)
            nc.vector.tensor_copy(out=pools[:], in_=poolp[:])
            nc.sync.dma_start(out=pool_in.ap(), in_=pools[:])
            nc.gpsimd.collective_compute(
                "AllReduce", OP.add, replica_groups=RG8,
                ins=[pool_in.ap()], outs=[pool_out.ap()])

            # ---- head ----
            pooled = cst.tile([F + 1, N_GRAPHS], F32)
            nc.sync.dma_start(out=pooled[:], in_=pool_out.ap())
            poolb = cst.tile([F + 1, N_GRAPHS], BF)
            nc.vector.tensor_copy(out=poolb[:], in_=pooled[:])
            zp = pc.tile([4, N_GRAPHS], F32, space="PSUM", tag="z")
            nc.tensor.matmul(out=zp[:], lhsT=metat[:F + 1, 452:456],
                             rhs=poolb[:], start=True, stop=True)
            zs = cst.tile([4, N_GRAPHS], F32)
            nc.vector.tensor_copy(out=zs[:], in_=zp[:])
            identf = cst.tile([P, P], F32)
            make_identity(nc, identf[:])
            ztp = pc.tile([N_GRAPHS, 4], F32, space="PSUM", tag="zt")
            nc.tensor.transpose(out=ztp[:], in_=zs[:], identity=identf[:4, :4])
            zt = cst.tile([N_GRAPHS, 4], F32)
            nc.vector.tensor_copy(out=zt[:], in_=ztp[:])
            rc = cst.tile([N_GRAPHS, 1], F32)
            nc.vector.reciprocal(out=rc[:], in_=zt[:, 3:4])
            lg = cst.tile([N_GRAPHS, N_ACT], F32)
            nc.vector.tensor_tensor(out=lg[:], in0=zt[:, :N_ACT],
                                    in1=rc[:].to_broadcast([N_GRAPHS, N_ACT]),
                                    op=OP.mult)
            mx = cst.tile([N_GRAPHS, 1], F32)
            nc.vector.tensor_reduce(out=mx[:], in_=lg[:], op=OP.max,
                                    axis=mybir.AxisListType.X)
            nc.vector.tensor_tensor(
                out=lg[:], in0=lg[:],
                in1=mx[:].to_broadcast([N_GRAPHS, N_ACT]), op=OP.subtract)
            nc.scalar.activation(lg[:], lg[:], AF.Exp)
            sm = cst.tile([N_GRAPHS, 1], F32)
            nc.vector.tensor_reduce(out=sm[:], in_=lg[:], op=OP.add,
                                    axis=mybir.AxisListType.X)
            nc.vector.reciprocal(out=sm[:], in_=sm[:])
            nc.vector.tensor_tensor(
                out=lg[:], in0=lg[:],
                in1=sm[:].to_broadcast([N_GRAPHS, N_ACT]), op=OP.mult)
            nc.sync.dma_start(out=out_h.ap(), in_=lg[:])

    nc.compile()
    return nc


def kernel(x, edge_index, batch, W1, b1, W2, b2, Wl, bl):
    from concourse.bass_utils import run_bass_kernel_spmd
    in_maps, calls, chunks = _prep(np.asarray(x), np.asarray(edge_index),
                                   np.asarray(batch), np.asarray(W1),
                                   np.asarray(b1), np.asarray(W2),
                                   np.asarray(b2), np.asarray(Wl),
                                   np.asarray(bl))
    nc = _build(calls, chunks)
    res = run_bass_kernel_spmd(nc, in_maps, core_ids=list(range(8)))
    return np.asarray(res.results[0]["out"], dtype=np.float32)


# revision 46
# speedup vs baseline: 3.1225x; 1.1307x over previous
"""2-layer GCN (GridGNN) on 8 Trainium2 NeuronCores.

2D sharding: core c=(q,h), q=c//2 source-quarter (25088 nodes), h=c%2
destination parity group. Core c handles edges with src in quarter q and
dst in shards {s: s%2==h}. Each core ships only its OWN shard of x (fp8,
~0.8MB); the per-quarter message table is built on-device by transforming
the own shard and AllGathering within quarter pairs, then cast-DMA'd to a
flat f32 table in HBM (trailing zero row as gather-padding target).
Messages move via gpsimd dma_gather (source-node indices) and accumulate
into the f32 partial-aggregate HBM buffer via dma_scatter_add. The SDMA
CCE += loses updates when a call contains duplicate destination rows
(verified on HW), so edges are ranked host-side by occurrence number
within their destination row and emitted as one gather+scatter call per
(dst-half, rank, piece) — rows within a call are then unique, and calls
targeting the same half are serialized by the tile framework's WAW
semaphores. Padding tokens gather the zero row and scatter onto a
zero-degree row, so they cannot race real updates. Partials are
ReduceScattered within parity groups; pooled sums (count-augmented via a
homogeneous column) are AllReduced; linear+softmax head on device. All
small per-core constants ship as one packed bf16 tensor to minimize
per-array transfer round-trips over the axon tunnel.
"""
import numpy as np
import ml_dtypes

N_NODES = 100000
N_GRAPHS = 64
F = 64
N_ACT = 3
P = 128
SHARD = 12544
NW = 98
QUART = 2 * SHARD
ZROW = QUART          # zero row appended to the message table
NWIN = 4 * NW
CHUNK_W = 14
HALF = 2 * SHARD      # rows per scatter half-region of rs_in
TCALL = 6272          # max tokens per gather/scatter call
MCOLS = 456           # packed meta tensor columns
RMUL = 1 << 20        # (half, rank) sort-key multiplier

bf16 = ml_dtypes.bfloat16
f8e4 = ml_dtypes.float8_e4m3


def _prep(x, edge_index, batch, W1, b1, W2, b2, Wl, bl):
    src = edge_index[0].astype(np.int64)
    dst = edge_index[1].astype(np.int64)
    q_e = src // QUART
    shard_e = dst // SHARD
    core_e = q_e * 2 + (shard_e % 2)

    # Per core: split edges by dst half (2 shard-slots each), rank each edge
    # by its occurrence number within its destination row so that every
    # (half, rank) slice has unique rows -> dma_scatter_add is exact.
    per_core = []          # (gi, rowh, half, rank) arrays, edges sorted
    cnt_hr = {}            # (c, half) -> array of per-rank counts
    trash = np.zeros((8, 2), np.int64)
    for c in range(8):
        m = core_e == c
        s, d = src[m], dst[m]
        sh = d // SHARD
        slot = sh // 2                     # 0..3 within parity group
        dlocal = d - sh * SHARD
        row = slot * SHARD + dlocal        # row in rs_in [4*SHARD]
        half = slot // 2
        rowh = row - half * HALF           # row within half [0, HALF)
        gi = s - (c // 2) * QUART
        # occurrence rank of each edge within (half, rowh)
        key = half * HALF + rowh
        order = np.argsort(key, kind="stable")
        ks = key[order]
        starts = np.r_[0, np.nonzero(np.diff(ks))[0] + 1]
        reps = np.diff(np.r_[starts, ks.size])
        rank_sorted = np.arange(ks.size) - np.repeat(starts, reps)
        rank = np.empty(ks.size, np.int64)
        rank[order] = rank_sorted
        per_core.append((gi, rowh, half, rank))
        for hf in range(2):
            mh = half == hf
            cnt_hr[(c, hf)] = np.bincount(rank[mh]) if mh.any() else \
                np.zeros(1, np.int64)
            # a row with no edges at all in this half (pad target)
            used = np.zeros(HALF, bool)
            used[rowh[mh]] = True
            free = np.nonzero(~used)[0]
            assert free.size > 0, "no zero-degree row in half"
            trash[c, hf] = free[0]

    # Call schedule, identical across cores. Rank 0 covers nearly every row,
    # so it is emitted DENSE: for each half, 4 calls of TCALL tokens whose
    # scatter index is implicit (token i -> row r0+i) — a plain DMA write
    # that also zero-initializes rows with no rank-0 edge (they gather the
    # zero row). Ranks >= 1 stay sparse with explicit (unique) rows.
    # calls: (kind, half, rank, size, piece); dense piece j covers rows
    # [j*TCALL, (j+1)*TCALL).
    assert HALF % TCALL == 0
    calls = []
    for hf in range(2):
        for j in range(HALF // TCALL):
            calls.append(("d", hf, 0, TCALL, j))
        rmax = max(len(cnt_hr[(c, hf)]) for c in range(8))
        for r in range(1, rmax):
            mx = max(int(cnt_hr[(c, hf)][r]) if r < len(cnt_hr[(c, hf)])
                     else 0 for c in range(8))
            left, j = mx, 0
            while left > 0:
                sz = -(-min(TCALL, left) // P) * P
                calls.append(("s", hf, r, sz, j))
                left -= TCALL
                j += 1

    Etot_g = sum(sz for (_, _, _, sz, _) in calls)
    Etot_s = sum(sz for (k, _, _, sz, _) in calls if k == "s")
    goffs, soffs, go, so = [], [], 0, 0
    chunks = []                  # (kind, hf, ga, gb, sa, sb) sa=row0 if dense
    for (k, hf, r, sz, j) in calls:
        if k == "d":
            chunks.append(("d", hf, go, go + sz, j * TCALL, 0))
        else:
            chunks.append(("s", hf, go, go + sz, so, so + sz))
            so += sz
        goffs.append(go)
        go += sz

    gkeys = np.array([hf * RMUL + r for (k, hf, r, _, _) in calls])
    gidx_all = np.full((8, Etot_g), ZROW, np.int16)
    sidx_all = np.zeros((8, Etot_s), np.int16)
    for c in range(8):
        gi, rowh, half, rank = per_core[c]
        # dense rank-0 tables per half
        dense = np.full((2, HALF), ZROW, np.int16)
        m0 = rank == 0
        dense[half[m0], rowh[m0]] = gi[m0]
        # sort edges by (half, rank, rowh) for deterministic sparse packing
        gkey = half * RMUL + rank
        order = np.argsort(gkey * np.int64(HALF) + rowh, kind="stable")
        gi, rowh, gkey = gi[order], rowh[order], gkey[order]
        g0 = np.searchsorted(gkey, gkeys, side="left")
        g1 = np.searchsorted(gkey, gkeys, side="right")
        for i, (k, hf, r, sz, j) in enumerate(calls):
            ga = goffs[i]
            if k == "d":
                gidx_all[c, ga:ga + sz] = dense[hf, j * TCALL:(j + 1) * TCALL]
                continue
            sa = chunks[i][4]
            sidx_all[c, sa:sa + sz] = trash[c, hf]
            s0 = g0[i] + j * TCALL
            n = min(int(g1[i]) - s0, sz)
            if n > 0:
                gidx_all[c, ga:ga + n] = gi[s0:s0 + n]
                sidx_all[c, sa:sa + n] = rowh[s0:s0 + n]

    # wrap in 16 partitions (token t at [t%16, t//16]), per call
    def wrap16(v_all, spans):
        out = []
        for c in range(8):
            cols = [v_all[c, a:b].reshape(-1, 16).T for (a, b) in spans]
            out.append(np.concatenate(cols, axis=1))
        return np.stack(out)
    gidx_sb = wrap16(gidx_all, [(ch[2], ch[3]) for ch in chunks])
    sidx_sb = wrap16(sidx_all, [(ch[4], ch[5]) for ch in chunks
                                if ch[0] == "s"])

    deg = np.zeros(8 * SHARD, np.int64)
    np.add.at(deg, dst, 1)
    xpad = np.zeros((8 * SHARD, F), np.float32)
    xpad[:N_NODES] = x
    bpad = np.full(8 * SHARD, 127, np.float32)
    bpad[:N_NODES] = batch

    in_maps = []
    for c in range(8):
        os_ = slice(c * SHARD, (c + 1) * SHARD)
        meta = np.zeros((P, MCOLS), np.float32)
        meta[:, 0:NW] = deg[os_].reshape(NW, P).T
        meta[:, NW:2 * NW] = bpad[os_].reshape(NW, P).T
        meta[:, 196:260] = np.broadcast_to(b1, (P, F))
        meta[:, 260:324] = np.broadcast_to(b2, (P, F))
        meta[:F, 324:388] = W1
        meta[:, 388:452] = np.concatenate([W2, W2], axis=0)
        meta[:F + 1, 452:456] = _wl_aug(Wl, bl)
        in_maps.append({
            "xo_T": np.ascontiguousarray(xpad[os_].T.astype(f8e4)),
            "idx": np.ascontiguousarray(
                np.concatenate([gidx_sb[c], sidx_sb[c]], axis=1)),
            "meta": meta.astype(bf16),
        })
    return in_maps, calls, chunks


def _wl_aug(Wl, bl):
    Wl_aug = np.zeros((F + 1, 4), np.float32)
    Wl_aug[:F, :3] = Wl
    Wl_aug[F, :3] = bl
    Wl_aug[F, 3] = 1.0
    return Wl_aug


def _build(calls, chunks):
    import concourse.bass as bass
    import concourse.bacc as bacc
    import concourse.tile as tile
    import concourse.mybir as mybir
    from concourse.library_config import mlp
    from concourse.masks import make_identity

    Etot_g = max(ch[3] for ch in chunks)
    Etot_s = max((ch[5] for ch in chunks if ch[0] == "s"), default=0)
    nc = bacc.Bacc("TRN2", target_bir_lowering=False, debug=False,
                   num_devices=8)
    F32, BF, I16 = mybir.dt.float32, mybir.dt.bfloat16, mybir.dt.int16
    F8 = mybir.dt.float8e4
    AF = mybir.ActivationFunctionType
    OP = mybir.AluOpType

    def ein(name, shape, dt):
        return nc.dram_tensor(name, shape, dt, kind="ExternalInput")

    xo_T = ein("xo_T", [F, SHARD], F8)
    idxh = ein("idx", [16, (Etot_g + Etot_s) // 16], I16)
    metah = ein("meta", [P, MCOLS], BF)
    out_h = nc.dram_tensor("out", [N_GRAPHS, N_ACT], F32,
                           kind="ExternalOutput")

    ftab = [nc.dram_tensor(f"ftab{i}", [QUART + P, F], F32, kind="Internal")
            for i in range(2)]
    rs_in = [nc.dram_tensor(f"rs_in{i}", [4 * SHARD, F], F32, kind="Internal")
             for i in range(2)]
    rs_out = [nc.dram_tensor(f"rs_out{i}", [SHARD, F], F32, kind="Internal")
              for i in range(2)]
    ag_in = [nc.dram_tensor(f"ag_in{i}", [SHARD, F], BF, kind="Internal")
             for i in range(2)]
    ag_out = [nc.dram_tensor(f"ag_out{i}", [QUART, F], BF, kind="Internal")
              for i in range(2)]
    pool_in = nc.dram_tensor("pool_in", [F + 1, N_GRAPHS], F32,
                             kind="Internal")
    pool_out = nc.dram_tensor("pool_out", [F + 1, N_GRAPHS], F32,
                              kind="Internal", addr_space="Shared")

    RG2 = [[0, 1], [2, 3], [4, 5], [6, 7]]
    RGH = [[0, 2, 4, 6], [1, 3, 5, 7]]
    RG8 = [[0, 1, 2, 3, 4, 5, 6, 7]]

    nc.gpsimd.load_library(mlp)
    with tile.TileContext(nc) as tc:
        with tc.tile_pool(name="cst", bufs=1) as cst, \
             tc.tile_pool(name="big", bufs=1) as big, \
             tc.tile_pool(name="mv", bufs=2) as mv, \
             tc.tile_pool(name="ps", bufs=2, space="PSUM") as ps, \
             tc.tile_pool(name="pw", bufs=2, space="PSUM") as pw, \
             tc.tile_pool(name="pc", bufs=1, space="PSUM") as pc:

            ident = cst.tile([P, P], BF)
            make_identity(nc, ident[:])
            iota_i = cst.tile([P, N_GRAPHS], mybir.dt.int32)
            nc.gpsimd.iota(iota_i[:], pattern=[[1, N_GRAPHS]], base=0,
                           channel_multiplier=0)
            iota = cst.tile([P, N_GRAPHS], BF)
            nc.vector.tensor_copy(out=iota[:], in_=iota_i[:])

            metat = cst.tile([P, MCOLS], BF)
            nc.sync.dma_start(out=metat[:], in_=metah.ap())
            batt = metat[:, NW:2 * NW]
            b1t = metat[:, 196:260]
            b2t = metat[:, 260:324]
            W1t = metat[:F, 324:388]
            W2t = metat[:, 388:452]
            # replicate compact idx lists across the 8 channel groups
            idxg = cst.tile([P, Etot_g // 16], I16)
            idxs = cst.tile([P, Etot_s // 16], I16)
            gcols = Etot_g // 16
            for k in range(8):
                nc.sync.dma_start(out=idxg[16 * k:16 * (k + 1), :],
                                  in_=idxh.ap()[:, :gcols])
                nc.sync.dma_start(out=idxs[16 * k:16 * (k + 1), :],
                                  in_=idxh.ap()[:, gcols:])

            zC = cst.tile([P, F], F32)
            nc.vector.memset(zC[:], 0.0)
            # zero rows ZROW..ZROW+P of both message tables (padding target)
            for li in range(2):
                nc.sync.dma_start(out=ftab[li].ap()[ZROW:ZROW + P, :],
                                  in_=zC[:])

            dinvo = cst.tile([P, NW], F32)
            nc.vector.tensor_copy(out=dinvo[:], in_=metat[:, :NW])
            nc.vector.tensor_scalar(out=dinvo[:], in0=dinvo[:], scalar1=1.0,
                                    scalar2=None, op0=OP.add)
            nc.vector.reciprocal(out=dinvo[:], in_=dinvo[:])
            nc.scalar.activation(dinvo[:], dinvo[:], AF.Sqrt)
            dvb = dinvo[:].unsqueeze(2).to_broadcast([P, NW, F])

            tso = big.tile([P, NW * F], BF)      # (x@W1)*dinv, own shard
            h1own = big.tile([P, NW * F], BF)
            self2 = big.tile([P, NW * F], BF)
            ts2all = big.tile([P, NW * F], BF)
            h2aug = big.tile([P, NW * (F + 1)], BF)
            agg = big.tile([P, NW * F], BF)

            tso3 = tso[:].rearrange("p (t f) -> p t f", f=F)

            # ---- layer 1 transform (own shard), streamed ----
            XC = 14
            for t0 in range(0, NW, XC):
                t1 = min(t0 + XC, NW)
                xc8 = mv.tile([F, XC * P], F8, tag="xc8")
                nc.sync.dma_start(out=xc8[:, :(t1 - t0) * P],
                                  in_=xo_T.ap()[:, t0 * P:t1 * P])
                xc = mv.tile([F, XC * P], BF, tag="xc")
                nc.vector.tensor_copy(out=xc[:, :(t1 - t0) * P],
                                      in_=xc8[:, :(t1 - t0) * P])
                for t in range(t0, t1):
                    pt = pw.tile([P, F], F32, space="PSUM", tag="tr")
                    nc.tensor.matmul(
                        out=pt[:], lhsT=xc[:, (t - t0) * P:(t - t0 + 1) * P],
                        rhs=W1t, start=True, stop=True)
                    nc.vector.tensor_tensor(
                        out=tso3[:, t, :], in0=pt[:],
                        in1=dinvo[:, t:t + 1].to_broadcast([P, F]),
                        op=OP.mult)
            nc.sync.dma_start(
                out=ag_in[0].ap().rearrange("(w p) f -> p w f", p=P),
                in_=tso3)
            nc.gpsimd.collective_compute(
                "AllGather", OP.bypass, replica_groups=RG2,
                ins=[ag_in[0].ap()], outs=[ag_out[0].ap()])
            nc.gpsimd.dma_start(out=ftab[0].ap()[:QUART, :],
                                in_=ag_out[0].ap())

            MSZ = TCALL // P

            def edge_phase(li):
                for (kind, hf, ga, gb, sa, sb) in chunks:
                    nt = (gb - ga) // P
                    msg = mv.tile([P, MSZ * F], F32, tag="msg")
                    nc.gpsimd.dma_gather(
                        out_ap=msg[:, :nt * F].rearrange(
                            "p (t f) -> p t f", f=F),
                        in_ap=ftab[li].ap(),
                        idxs_ap=idxg[:, ga // 16:gb // 16],
                        num_idxs=gb - ga,
                        num_idxs_reg=gb - ga,
                        elem_size=F,
                        single_packet=False,
                    )
                    if kind == "d":
                        r0 = hf * HALF + sa
                        nc.sync.dma_start(
                            out=rs_in[li].ap()[r0:r0 + (gb - ga), :]
                                .rearrange("(t p) f -> p t f", p=P),
                            in_=msg[:, :nt * F].rearrange(
                                "p (t f) -> p t f", f=F))
                    else:
                        nc.gpsimd.dma_scatter_add(
                            out_ap=rs_in[li].ap()[hf * HALF:(hf + 1) * HALF,
                                                  :],
                            in_ap=msg[:, :nt * F].rearrange(
                                "p (t f) -> p t f", f=F),
                            idxs_ap=idxs[:, sa // 16:sb // 16],
                            num_idxs=gb - ga,
                            num_idxs_reg=gb - ga,
                            elem_size=F,
                        )
                nc.gpsimd.collective_compute(
                    "ReduceScatter", OP.add, replica_groups=RGH,
                    ins=[rs_in[li].ap()], outs=[rs_out[li].ap()])

            def load_agg(li):
                a3 = agg[:].rearrange("p (w f) -> p w f", f=F)
                for w0 in range(0, NW, CHUNK_W):
                    w1 = min(w0 + CHUNK_W, NW)
                    ar = mv.tile([P, CHUNK_W * F], F32, tag="ar")
                    nc.sync.dma_start(
                        out=ar[:, :(w1 - w0) * F].rearrange(
                            "p (w f) -> p w f", f=F),
                        in_=rs_out[li].ap()[w0 * P:w1 * P, :].rearrange(
                            "(w p) f -> p w f", p=P))
                    nc.vector.tensor_copy(
                        out=a3[:, w0:w1, :],
                        in_=ar[:, :(w1 - w0) * F].rearrange(
                            "p (w f) -> p w f", f=F))
                return a3

            # ---- layer 1 ----
            edge_phase(0)
            a3 = load_agg(0)
            h3 = h1own[:].rearrange("p (w f) -> p w f", f=F)
            # h1 = relu((agg + tso) * dinv + b1)
            nc.vector.tensor_tensor(out=h3[:], in0=a3[:], in1=tso3[:],
                                    op=OP.add)
            nc.vector.tensor_tensor(out=h3[:], in0=h3[:], in1=dvb,
                                    op=OP.mult)
            nc.vector.tensor_tensor(
                out=h3[:], in0=h3[:],
                in1=b1t.unsqueeze(1).to_broadcast([P, NW, F]), op=OP.add)
            nc.vector.tensor_scalar(out=h1own[:], in0=h1own[:],
                                    scalar1=0.0, scalar2=None, op0=OP.max)

            # ---- layer 2 transform (own shard): pairs of windows ----
            t23 = ts2all[:].rearrange("p (w f) -> p w f", f=F)
            for wp in range(0, NW, 2):
                trp = pc.tile([P, P], BF, space="PSUM", tag="trp")
                nc.tensor.transpose(out=trp[:],
                                    in_=h1own[:, wp * F:(wp + 2) * F],
                                    identity=ident[:])
                h1T = mv.tile([P, P], BF, tag="h1T")
                nc.vector.tensor_copy(out=h1T[:], in_=trp[:])
                for j in range(2):
                    w = wp + j
                    pt = pw.tile([P, F], F32, space="PSUM", tag="tr")
                    nc.tensor.matmul(out=pt[:], lhsT=h1T[j * F:(j + 1) * F, :],
                                     rhs=metat[j * F:(j + 1) * F, 388:452],
                                     start=True, stop=True)
                    nc.vector.tensor_tensor(
                        out=t23[:, w, :], in0=pt[:],
                        in1=dinvo[:, w:w + 1].to_broadcast([P, F]),
                        op=OP.mult)
            s23 = self2[:].rearrange("p (w f) -> p w f", f=F)
            nc.vector.tensor_tensor(out=s23[:], in0=t23[:], in1=dvb,
                                    op=OP.mult)
            nc.sync.dma_start(
                out=ag_in[1].ap().rearrange("(w p) f -> p w f", p=P),
                in_=t23)
            nc.gpsimd.collective_compute(
                "AllGather", OP.bypass, replica_groups=RG2,
                ins=[ag_in[1].ap()], outs=[ag_out[1].ap()])
            nc.gpsimd.dma_start(out=ftab[1].ap()[:QUART, :],
                                in_=ag_out[1].ap())

            # ---- layer 2 ----
            edge_phase(1)
            a23 = load_agg(1)
            h2a3 = h2aug[:].rearrange("p (w g) -> p w g", g=F + 1)
            nc.vector.memset(h2aug[:], 1.0)
            h2f = h2a3[:, :, :F]
            nc.vector.tensor_tensor(out=h2f, in0=a23[:], in1=dvb, op=OP.mult)
            nc.vector.tensor_tensor(out=h2f, in0=h2f, in1=s23[:], op=OP.add)
            nc.vector.tensor_tensor(
                out=h2f, in0=h2f,
                in1=b2t.unsqueeze(1).to_broadcast([P, NW, F]), op=OP.add)

            # ---- pooling ----
            ohg = big.tile([P, NW * N_GRAPHS], BF)
            nc.vector.tensor_tensor(
                out=ohg[:].rearrange("p (w g) -> p w g", g=N_GRAPHS),
                in0=batt.unsqueeze(2).to_broadcast([P, NW, N_GRAPHS]),
                in1=iota[:].unsqueeze(1).to_broadcast([P, NW, N_GRAPHS]),
                op=OP.is_equal)
            poolp = pc.tile([F + 1, N_GRAPHS], F32, space="PSUM", tag="pool")
            for w in range(NW):
                nc.tensor.matmul(out=poolp[:], lhsT=h2a3[:, w, :],
                                 rhs=ohg[:, w * N_GRAPHS:(w + 1) * N_GRAPHS],
                                 start=(w == 0), stop=(w == NW - 1))
            pools = cst.tile([F + 1, N_GRAPHS], F32)
            nc.vector.tensor_copy(out=pools[:], in_=poolp[:])
            nc.sync.dma_start(out=pool_in.ap(), in_=pools[:])
            nc.gpsimd.collective_compute(
                "AllReduce", OP.add, replica_groups=RG8,
                ins=[pool_in.ap()], outs=[pool_out.ap()])

            # ---- head ----
            pooled = cst.tile([F + 1, N_GRAPHS], F32)
            nc.sync.dma_start(out=pooled[:], in_=pool_out.ap())
            poolb = cst.tile([F + 1, N_GRAPHS], BF)
            nc.vector.tensor_copy(out=poolb[:], in_=pooled[:])
            zp = pc.tile([4, N_GRAPHS], F32, space="PSUM", tag="z")
            nc.tensor.matmul(out=zp[:], lhsT=metat[:F + 1, 452:456],
                             rhs=poolb[:], start=True, stop=True)
            zs = cst.tile([4, N_GRAPHS], F32)
            nc.vector.tensor_copy(out=zs[:], in_=zp[:])
            identf = cst.tile([P, P], F32)
            make_identity(nc, identf[:])
            ztp = pc.tile([N_GRAPHS, 4], F32, space="PSUM", tag="zt")
            nc.tensor.transpose(out=ztp[:], in_=zs[:], identity=identf[:4, :4])
            zt = cst.tile([N_GRAPHS, 4], F32)
            nc.vector.tensor_copy(out=zt[:], in_=ztp[:])
            rc = cst.tile([N_GRAPHS, 1], F32)
            nc.vector.reciprocal(out=rc[:], in_=zt[:, 3:4])
            lg = cst.tile([N_GRAPHS, N_ACT], F32)
            nc.vector.tensor_tensor(out=lg[:], in0=zt[:, :N_ACT],
                                    in1=rc[:].to_broadcast([N_GRAPHS, N_ACT]),
                                    op=OP.mult)
            mx = cst.tile([N_GRAPHS, 1], F32)
            nc.vector.tensor_reduce(out=mx[:], in_=lg[:], op=OP.max,
                                    axis=mybir.AxisListType.X)
            nc.vector.tensor_tensor(
                out=lg[:], in0=lg[:],
                in1=mx[:].to_broadcast([N_GRAPHS, N_ACT]), op=OP.subtract)
            nc.scalar.activation(lg[:], lg[:], AF.Exp)
            sm = cst.tile([N_GRAPHS, 1], F32)
            nc.vector.tensor_reduce(out=sm[:], in_=lg[:], op=OP.add,
                                    axis=mybir.AxisListType.X)
            nc.vector.reciprocal(out=sm[:], in_=sm[:])
            nc.vector.tensor_tensor(
                out=lg[:], in0=lg[:],
                in1=sm[:].to_broadcast([N_GRAPHS, N_ACT]), op=OP.mult)
            nc.sync.dma_start(out=out_h.ap(), in_=lg[:])

    nc.compile()
    return nc


def kernel(x, edge_index, batch, W1, b1, W2, b2, Wl, bl):
    from concourse.bass_utils import run_bass_kernel_spmd
    in_maps, calls, chunks = _prep(np.asarray(x), np.asarray(edge_index),
                                   np.asarray(batch), np.asarray(W1),
                                   np.asarray(b1), np.asarray(W2),
                                   np.asarray(b2), np.asarray(Wl),
                                   np.asarray(bl))
    nc = _build(calls, chunks)
    res = run_bass_kernel_spmd(nc, in_maps, core_ids=list(range(8)))
    return np.asarray(res.results[0]["out"], dtype=np.float32)


# revision 47
# speedup vs baseline: 3.2685x; 1.0468x over previous
"""2-layer GCN (GridGNN) on 8 Trainium2 NeuronCores.

2D sharding: core c=(q,h), q=c//2 source-quarter (25088 nodes), h=c%2
destination parity group. Core c handles edges with src in quarter q and
dst in shards {s: s%2==h}. Each core ships only its OWN shard of x (fp8,
~0.8MB); the per-quarter message table is built on-device by transforming
the own shard and AllGathering within quarter pairs, then cast-DMA'd to a
flat f32 table in HBM (trailing zero row as gather-padding target).
Messages move via gpsimd dma_gather (source-node indices) and accumulate
into the f32 partial-aggregate HBM buffer via dma_scatter_add. The SDMA
CCE += loses updates when a call contains duplicate destination rows
(verified on HW), so edges are ranked host-side by occurrence number
within their destination row and emitted as one gather+scatter call per
(dst-half, rank, piece) — rows within a call are then unique, and calls
targeting the same half are serialized by the tile framework's WAW
semaphores. Padding tokens gather the zero row and scatter onto a
zero-degree row, so they cannot race real updates. Partials are
ReduceScattered within parity groups; pooled sums (count-augmented via a
homogeneous column) are AllReduced; linear+softmax head on device. All
small per-core constants ship as one packed bf16 tensor to minimize
per-array transfer round-trips over the axon tunnel.
"""
import numpy as np
import ml_dtypes

N_NODES = 100000
N_GRAPHS = 64
F = 64
N_ACT = 3
P = 128
SHARD = 12544
NW = 98
QUART = 2 * SHARD
ZROW = QUART          # zero row appended to the message table
NWIN = 4 * NW
CHUNK_W = 14
HALF = 2 * SHARD      # rows per scatter half-region of rs_in
TCALL = 6272          # max tokens per gather/scatter call
MCOLS = 456           # packed meta tensor columns
RMUL = 1 << 20        # (half, rank) sort-key multiplier

bf16 = ml_dtypes.bfloat16
f8e4 = ml_dtypes.float8_e4m3


def _prep(x, edge_index, batch, W1, b1, W2, b2, Wl, bl):
    src = edge_index[0].astype(np.int64)
    dst = edge_index[1].astype(np.int64)
    q_e = src // QUART
    shard_e = dst // SHARD
    core_e = q_e * 2 + (shard_e % 2)

    # Per core: split edges by dst half (2 shard-slots each), rank each edge
    # by its occurrence number within its destination row so that every
    # (half, rank) slice has unique rows -> dma_scatter_add is exact.
    per_core = []          # (gi, rowh, half, rank) arrays, edges sorted
    cnt_hr = {}            # (c, half) -> array of per-rank counts
    trash = np.zeros((8, 2), np.int64)
    for c in range(8):
        m = core_e == c
        s, d = src[m], dst[m]
        sh = d // SHARD
        slot = sh // 2                     # 0..3 within parity group
        dlocal = d - sh * SHARD
        row = slot * SHARD + dlocal        # row in rs_in [4*SHARD]
        half = slot // 2
        rowh = row - half * HALF           # row within half [0, HALF)
        gi = s - (c // 2) * QUART
        # occurrence rank of each edge within (half, rowh)
        key = half * HALF + rowh
        order = np.argsort(key, kind="stable")
        ks = key[order]
        starts = np.r_[0, np.nonzero(np.diff(ks))[0] + 1]
        reps = np.diff(np.r_[starts, ks.size])
        rank_sorted = np.arange(ks.size) - np.repeat(starts, reps)
        rank = np.empty(ks.size, np.int64)
        rank[order] = rank_sorted
        per_core.append((gi, rowh, half, rank))
        for hf in range(2):
            mh = half == hf
            cnt_hr[(c, hf)] = np.bincount(rank[mh]) if mh.any() else \
                np.zeros(1, np.int64)
            # a row with no edges at all in this half (pad target)
            used = np.zeros(HALF, bool)
            used[rowh[mh]] = True
            free = np.nonzero(~used)[0]
            assert free.size > 0, "no zero-degree row in half"
            trash[c, hf] = free[0]

    # Call schedule, identical across cores. Rank 0 covers nearly every row,
    # so it is emitted DENSE: for each half, 4 calls of TCALL tokens whose
    # scatter index is implicit (token i -> row r0+i) — a plain DMA write
    # that also zero-initializes rows with no rank-0 edge (they gather the
    # zero row). Ranks >= 1 stay sparse with explicit (unique) rows.
    # calls: (kind, half, rank, size, piece); dense piece j covers rows
    # [j*TCALL, (j+1)*TCALL).
    assert HALF % TCALL == 0
    calls = []
    for hf in range(2):
        for j in range(HALF // TCALL):
            calls.append(("d", hf, 0, TCALL, j))
        rmax = max(len(cnt_hr[(c, hf)]) for c in range(8))
        for r in range(1, rmax):
            mx = max(int(cnt_hr[(c, hf)][r]) if r < len(cnt_hr[(c, hf)])
                     else 0 for c in range(8))
            left, j = mx, 0
            while left > 0:
                sz = -(-min(TCALL, left) // P) * P
                calls.append(("s", hf, r, sz, j))
                left -= TCALL
                j += 1

    Etot_g = sum(sz for (_, _, _, sz, _) in calls)
    Etot_s = sum(sz for (k, _, _, sz, _) in calls if k == "s")
    goffs, soffs, go, so = [], [], 0, 0
    chunks = []                  # (kind, hf, ga, gb, sa, sb) sa=row0 if dense
    for (k, hf, r, sz, j) in calls:
        if k == "d":
            chunks.append(("d", hf, go, go + sz, j * TCALL, 0))
        else:
            chunks.append(("s", hf, go, go + sz, so, so + sz))
            so += sz
        goffs.append(go)
        go += sz

    gkeys = np.array([hf * RMUL + r for (k, hf, r, _, _) in calls])
    gidx_all = np.full((8, Etot_g), ZROW, np.int16)
    sidx_all = np.zeros((8, Etot_s), np.int16)
    for c in range(8):
        gi, rowh, half, rank = per_core[c]
        # dense rank-0 tables per half
        dense = np.full((2, HALF), ZROW, np.int16)
        m0 = rank == 0
        dense[half[m0], rowh[m0]] = gi[m0]
        # sort edges by (half, rank, rowh) for deterministic sparse packing
        gkey = half * RMUL + rank
        order = np.argsort(gkey * np.int64(HALF) + rowh, kind="stable")
        gi, rowh, gkey = gi[order], rowh[order], gkey[order]
        g0 = np.searchsorted(gkey, gkeys, side="left")
        g1 = np.searchsorted(gkey, gkeys, side="right")
        for i, (k, hf, r, sz, j) in enumerate(calls):
            ga = goffs[i]
            if k == "d":
                gidx_all[c, ga:ga + sz] = dense[hf, j * TCALL:(j + 1) * TCALL]
                continue
            sa = chunks[i][4]
            sidx_all[c, sa:sa + sz] = trash[c, hf]
            s0 = g0[i] + j * TCALL
            n = min(int(g1[i]) - s0, sz)
            if n > 0:
                gidx_all[c, ga:ga + n] = gi[s0:s0 + n]
                sidx_all[c, sa:sa + n] = rowh[s0:s0 + n]

    # wrap in 16 partitions (token t at [t%16, t//16]), per call
    def wrap16(v_all, spans):
        out = []
        for c in range(8):
            cols = [v_all[c, a:b].reshape(-1, 16).T for (a, b) in spans]
            out.append(np.concatenate(cols, axis=1))
        return np.stack(out)
    gidx_sb = wrap16(gidx_all, [(ch[2], ch[3]) for ch in chunks])
    sidx_sb = wrap16(sidx_all, [(ch[4], ch[5]) for ch in chunks
                                if ch[0] == "s"])

    deg = np.zeros(8 * SHARD, np.int64)
    np.add.at(deg, dst, 1)
    xpad = np.zeros((8 * SHARD, F), np.float32)
    xpad[:N_NODES] = x
    bpad = np.full(8 * SHARD, 127, np.float32)
    bpad[:N_NODES] = batch

    in_maps = []
    for c in range(8):
        os_ = slice(c * SHARD, (c + 1) * SHARD)
        meta = np.zeros((P, MCOLS), np.float32)
        meta[:, 0:NW] = deg[os_].reshape(NW, P).T
        meta[:, NW:2 * NW] = bpad[os_].reshape(NW, P).T
        meta[:, 196:260] = np.broadcast_to(b1, (P, F))
        meta[:, 260:324] = np.broadcast_to(b2, (P, F))
        meta[:F, 324:388] = W1
        meta[:, 388:452] = np.concatenate([W2, W2], axis=0)
        meta[:F + 1, 452:456] = _wl_aug(Wl, bl)
        in_maps.append({
            "xo_T": np.ascontiguousarray(xpad[os_].T.astype(f8e4)),
            "idx": np.ascontiguousarray(
                np.concatenate([gidx_sb[c], sidx_sb[c]], axis=1)),
            "meta": meta.astype(bf16),
        })
    return in_maps, calls, chunks


def _wl_aug(Wl, bl):
    Wl_aug = np.zeros((F + 1, 4), np.float32)
    Wl_aug[:F, :3] = Wl
    Wl_aug[F, :3] = bl
    Wl_aug[F, 3] = 1.0
    return Wl_aug


def _build(calls, chunks):
    import concourse.bass as bass
    import concourse.bacc as bacc
    import concourse.tile as tile
    import concourse.mybir as mybir
    from concourse.library_config import mlp
    from concourse.masks import make_identity

    Etot_g = max(ch[3] for ch in chunks)
    Etot_s = max((ch[5] for ch in chunks if ch[0] == "s"), default=0)
    nc = bacc.Bacc("TRN2", target_bir_lowering=False, debug=False,
                   num_devices=8)
    F32, BF, I16 = mybir.dt.float32, mybir.dt.bfloat16, mybir.dt.int16
    F8 = mybir.dt.float8e4
    AF = mybir.ActivationFunctionType
    OP = mybir.AluOpType

    def ein(name, shape, dt):
        return nc.dram_tensor(name, shape, dt, kind="ExternalInput")

    xo_T = ein("xo_T", [F, SHARD], F8)
    idxh = ein("idx", [16, (Etot_g + Etot_s) // 16], I16)
    metah = ein("meta", [P, MCOLS], BF)
    out_h = nc.dram_tensor("out", [N_GRAPHS, N_ACT], F32,
                           kind="ExternalOutput")

    ftab = [nc.dram_tensor(f"ftab{i}", [QUART + P, F], F32, kind="Internal")
            for i in range(2)]
    rs_in = [nc.dram_tensor(f"rs_in{i}", [4 * SHARD, F], F32, kind="Internal")
             for i in range(2)]
    rs_out = [nc.dram_tensor(f"rs_out{i}", [SHARD, F], F32, kind="Internal")
              for i in range(2)]
    ag_in = [nc.dram_tensor(f"ag_in{i}", [SHARD, F], BF, kind="Internal")
             for i in range(2)]
    ag_out = [nc.dram_tensor(f"ag_out{i}", [QUART, F], BF, kind="Internal")
              for i in range(2)]
    pool_in = nc.dram_tensor("pool_in", [F + 1, N_GRAPHS], F32,
                             kind="Internal")
    pool_out = nc.dram_tensor("pool_out", [F + 1, N_GRAPHS], F32,
                              kind="Internal", addr_space="Shared")

    RG2 = [[0, 1], [2, 3], [4, 5], [6, 7]]
    RGH = [[0, 2, 4, 6], [1, 3, 5, 7]]
    RG8 = [[0, 1, 2, 3, 4, 5, 6, 7]]

    nc.gpsimd.load_library(mlp)
    with tile.TileContext(nc) as tc:
        with tc.tile_pool(name="cst", bufs=1) as cst, \
             tc.tile_pool(name="big", bufs=1) as big, \
             tc.tile_pool(name="mv", bufs=2) as mv, \
             tc.tile_pool(name="ps", bufs=2, space="PSUM") as ps, \
             tc.tile_pool(name="pw", bufs=2, space="PSUM") as pw, \
             tc.tile_pool(name="pc", bufs=1, space="PSUM") as pc:

            ident = cst.tile([P, P], BF)
            make_identity(nc, ident[:])
            iota_i = cst.tile([P, N_GRAPHS], mybir.dt.int32)
            nc.gpsimd.iota(iota_i[:], pattern=[[1, N_GRAPHS]], base=0,
                           channel_multiplier=0)
            iota = cst.tile([P, N_GRAPHS], BF)
            nc.vector.tensor_copy(out=iota[:], in_=iota_i[:])

            metat = cst.tile([P, MCOLS], BF)
            nc.sync.dma_start(out=metat[:], in_=metah.ap())
            batt = metat[:, NW:2 * NW]
            b1t = metat[:, 196:260]
            b2t = metat[:, 260:324]
            W1t = metat[:F, 324:388]
            W2t = metat[:, 388:452]
            # replicate compact idx lists across the 8 channel groups
            idxg = cst.tile([P, Etot_g // 16], I16)
            idxs = cst.tile([P, Etot_s // 16], I16)
            gcols = Etot_g // 16
            for k in range(8):
                nc.sync.dma_start(out=idxg[16 * k:16 * (k + 1), :],
                                  in_=idxh.ap()[:, :gcols])
                nc.sync.dma_start(out=idxs[16 * k:16 * (k + 1), :],
                                  in_=idxh.ap()[:, gcols:])

            zC = cst.tile([P, F], F32)
            nc.vector.memset(zC[:], 0.0)
            # zero rows ZROW..ZROW+P of both message tables (padding target)
            for li in range(2):
                nc.sync.dma_start(out=ftab[li].ap()[ZROW:ZROW + P, :],
                                  in_=zC[:])

            dinvo = cst.tile([P, NW], F32)
            nc.vector.tensor_copy(out=dinvo[:], in_=metat[:, :NW])
            nc.vector.tensor_scalar(out=dinvo[:], in0=dinvo[:], scalar1=1.0,
                                    scalar2=None, op0=OP.add)
            nc.vector.reciprocal(out=dinvo[:], in_=dinvo[:])
            nc.scalar.activation(dinvo[:], dinvo[:], AF.Sqrt)
            dvb = dinvo[:].unsqueeze(2).to_broadcast([P, NW, F])

            tso = big.tile([P, NW * F], BF)      # (x@W1)*dinv, own shard
            h1own = big.tile([P, NW * F], BF)
            self2 = big.tile([P, NW * F], BF)
            ts2all = big.tile([P, NW * F], BF)
            h2aug = big.tile([P, NW * (F + 1)], BF)
            agg = big.tile([P, NW * F], BF)

            tso3 = tso[:].rearrange("p (t f) -> p t f", f=F)

            # ---- layer 1 transform (own shard), streamed ----
            XC = 14
            for t0 in range(0, NW, XC):
                t1 = min(t0 + XC, NW)
                xc8 = mv.tile([F, XC * P], F8, tag="xc8")
                nc.sync.dma_start(out=xc8[:, :(t1 - t0) * P],
                                  in_=xo_T.ap()[:, t0 * P:t1 * P])
                xc = mv.tile([F, XC * P], BF, tag="xc")
                nc.vector.tensor_copy(out=xc[:, :(t1 - t0) * P],
                                      in_=xc8[:, :(t1 - t0) * P])
                for t in range(t0, t1):
                    pt = pw.tile([P, F], F32, space="PSUM", tag="tr")
                    nc.tensor.matmul(
                        out=pt[:], lhsT=xc[:, (t - t0) * P:(t - t0 + 1) * P],
                        rhs=W1t, start=True, stop=True)
                    nc.vector.tensor_tensor(
                        out=tso3[:, t, :], in0=pt[:],
                        in1=dinvo[:, t:t + 1].to_broadcast([P, F]),
                        op=OP.mult)
            nc.sync.dma_start(
                out=ag_in[0].ap().rearrange("(w p) f -> p w f", p=P),
                in_=tso3)
            nc.gpsimd.collective_compute(
                "AllGather", OP.bypass, replica_groups=RG2,
                ins=[ag_in[0].ap()], outs=[ag_out[0].ap()])
            nc.gpsimd.dma_start(out=ftab[0].ap()[:QUART, :],
                                in_=ag_out[0].ap())

            MSZ = TCALL // P

            def edge_phase(li):
                for (kind, hf, ga, gb, sa, sb) in chunks:
                    nt = (gb - ga) // P
                    msg = mv.tile([P, MSZ * F], F32, tag="msg")
                    nc.gpsimd.dma_gather(
                        out_ap=msg[:, :nt * F].rearrange(
                            "p (t f) -> p t f", f=F),
                        in_ap=ftab[li].ap(),
                        idxs_ap=idxg[:, ga // 16:gb // 16],
                        num_idxs=gb - ga,
                        num_idxs_reg=gb - ga,
                        elem_size=F,
                        single_packet=False,
                    )
                    if kind == "d":
                        r0 = hf * HALF + sa
                        nc.sync.dma_start(
                            out=rs_in[li].ap()[r0:r0 + (gb - ga), :]
                                .rearrange("(t p) f -> p t f", p=P),
                            in_=msg[:, :nt * F].rearrange(
                                "p (t f) -> p t f", f=F))
                    else:
                        nc.gpsimd.dma_scatter_add(
                            out_ap=rs_in[li].ap()[hf * HALF:(hf + 1) * HALF,
                                                  :],
                            in_ap=msg[:, :nt * F].rearrange(
                                "p (t f) -> p t f", f=F),
                            idxs_ap=idxs[:, sa // 16:sb // 16],
                            num_idxs=gb - ga,
                            num_idxs_reg=gb - ga,
                            elem_size=F,
                        )
                nc.gpsimd.collective_compute(
                    "ReduceScatter", OP.add, replica_groups=RGH,
                    ins=[rs_in[li].ap()], outs=[rs_out[li].ap()])

            def load_agg(li):
                a3 = agg[:].rearrange("p (w f) -> p w f", f=F)
                for w0 in range(0, NW, CHUNK_W):
                    w1 = min(w0 + CHUNK_W, NW)
                    ar = mv.tile([P, CHUNK_W * F], F32, tag="ar")
                    nc.sync.dma_start(
                        out=ar[:, :(w1 - w0) * F].rearrange(
                            "p (w f) -> p w f", f=F),
                        in_=rs_out[li].ap()[w0 * P:w1 * P, :].rearrange(
                            "(w p) f -> p w f", p=P))
                    nc.vector.tensor_copy(
                        out=a3[:, w0:w1, :],
                        in_=ar[:, :(w1 - w0) * F].rearrange(
                            "p (w f) -> p w f", f=F))
                return a3

            # ---- layer 1 ----
            edge_phase(0)
            a3 = load_agg(0)
            h3 = h1own[:].rearrange("p (w f) -> p w f", f=F)
            # h1 = relu((agg + tso) * dinv + b1)
            nc.vector.tensor_tensor(out=h3[:], in0=a3[:], in1=tso3[:],
                                    op=OP.add)
            nc.vector.tensor_tensor(out=h3[:], in0=h3[:], in1=dvb,
                                    op=OP.mult)
            nc.vector.tensor_tensor(
                out=h3[:], in0=h3[:],
                in1=b1t.unsqueeze(1).to_broadcast([P, NW, F]), op=OP.add)
            nc.vector.tensor_scalar(out=h1own[:], in0=h1own[:],
                                    scalar1=0.0, scalar2=None, op0=OP.max)

            # ---- layer 2 transform (own shard): pairs of windows ----
            t23 = ts2all[:].rearrange("p (w f) -> p w f", f=F)
            for wp in range(0, NW, 2):
                trp = pc.tile([P, P], BF, space="PSUM", tag="trp")
                nc.tensor.transpose(out=trp[:],
                                    in_=h1own[:, wp * F:(wp + 2) * F],
                                    identity=ident[:])
                h1T = mv.tile([P, P], BF, tag="h1T")
                nc.vector.tensor_copy(out=h1T[:], in_=trp[:])
                for j in range(2):
                    w = wp + j
                    pt = pw.tile([P, F], F32, space="PSUM", tag="tr")
                    nc.tensor.matmul(out=pt[:], lhsT=h1T[j * F:(j + 1) * F, :],
                                     rhs=metat[j * F:(j + 1) * F, 388:452],
                                     start=True, stop=True)
                    nc.vector.tensor_tensor(
                        out=t23[:, w, :], in0=pt[:],
                        in1=dinvo[:, w:w + 1].to_broadcast([P, F]),
                        op=OP.mult)
            s23 = self2[:].rearrange("p (w f) -> p w f", f=F)
            nc.vector.tensor_tensor(out=s23[:], in0=t23[:], in1=dvb,
                                    op=OP.mult)
            nc.sync.dma_start(
                out=ag_in[1].ap().rearrange("(w p) f -> p w f", p=P),
                in_=t23)
            nc.gpsimd.collective_compute(
                "AllGather", OP.bypass, replica_groups=RG2,
                ins=[ag_in[1].ap()], outs=[ag_out[1].ap()])
            nc.gpsimd.dma_start(out=ftab[1].ap()[:QUART, :],
                                in_=ag_out[1].ap())

            # ---- layer 2 ----
            edge_phase(1)
            a23 = load_agg(1)
            h2a3 = h2aug[:].rearrange("p (w g) -> p w g", g=F + 1)
            nc.vector.memset(h2aug[:], 1.0)
            h2f = h2a3[:, :, :F]
            nc.vector.tensor_tensor(out=h2f, in0=a23[:], in1=dvb, op=OP.mult)
            nc.vector.tensor_tensor(out=h2f, in0=h2f, in1=s23[:], op=OP.add)
            nc.vector.tensor_tensor(
                out=h2f, in0=h2f,
                in1=b2t.unsqueeze(1).to_broadcast([P, NW, F]), op=OP.add)

            # ---- pooling ----
            ohg = big.tile([P, NW * N_GRAPHS], BF)
            nc.vector.tensor_tensor(
                out=ohg[:].rearrange("p (w g) -> p w g", g=N_GRAPHS),
                in0=batt.unsqueeze(2).to_broadcast([P, NW, N_GRAPHS]),
                in1=iota[:].unsqueeze(1).to_broadcast([P, NW, N_GRAPHS]),
                op=OP.is_equal)
            poolp = pc.tile([F + 1, N_GRAPHS], F32, space="PSUM", tag="pool")
            for w in range(NW):
                nc.tensor.matmul(out=poolp[:], lhsT=h2a3[:, w, :],
                                 rhs=ohg[:, w * N_GRAPHS:(w + 1) * N_GRAPHS],
                                 start=(w == 0), stop=(w == NW - 1))
            pools = cst.tile([F + 1, N_GRAPHS], F32)
            nc.vector.tensor_copy(out=pools[:], in_=poolp[:])
            nc.sync.dma_start(out=pool_in.ap(), in_=pools[:])
            nc.gpsimd.collective_compute(
                "AllReduce", OP.add, replica_groups=RG8,
                ins=[pool_in.ap()], outs=[pool_out.ap()])

            # ---- head ----
            pooled = cst.tile([F + 1, N_GRAPHS], F32)
            nc.sync.dma_start(out=pooled[:], in_=pool_out.ap())
            poolb = cst.tile([F + 1, N_GRAPHS], BF)
            nc.vector.tensor_copy(out=poolb[:], in_=pooled[:])
            zp = pc.tile([4, N_GRAPHS], F32, space="PSUM", tag="z")
            nc.tensor.matmul(out=zp[:], lhsT=metat[:F + 1, 452:456],
                             rhs=poolb[:], start=True, stop=True)
            zs = cst.tile([4, N_GRAPHS], F32)
            nc.vector.tensor_copy(out=zs[:], in_=zp[:])
            identf = cst.tile([P, P], F32)
            make_identity(nc, identf[:])
            ztp = pc.tile([N_GRAPHS, 4], F32, space="PSUM", tag="zt")
            nc.tensor.transpose(out=ztp[:], in_=zs[:], identity=identf[:4, :4])
            zt = cst.tile([N_GRAPHS, 4], F32)
            nc.vector.tensor_copy(out=zt[:], in_=ztp[:])
            rc = cst.tile([N_GRAPHS, 1], F32)
            nc.vector.reciprocal(out=rc[:], in_=zt[:, 3:4])
            lg = cst.tile([N_GRAPHS, N_ACT], F32)
            nc.vector.tensor_tensor(out=lg[:], in0=zt[:, :N_ACT],
                                    in1=rc[:].to_broadcast([N_GRAPHS, N_ACT]),
                                    op=OP.mult)
            mx = cst.tile([N_GRAPHS, 1], F32)
            nc.vector.tensor_reduce(out=mx[:], in_=lg[:], op=OP.max,
                                    axis=mybir.AxisListType.X)
            nc.vector.tensor_tensor(
                out=lg[:], in0=lg[:],
                in1=mx[:].to_broadcast([N_GRAPHS, N_ACT]), op=OP.subtract)
            nc.scalar.activation(lg[:], lg[:], AF.Exp)
            sm = cst.tile([N_GRAPHS, 1], F32)
            nc.vector.tensor_reduce(out=sm[:], in_=lg[:], op=OP.add,
                                    axis=mybir.AxisListType.X)
            nc.vector.reciprocal(out=sm[:], in_=sm[:])
            nc.vector.tensor_tensor(
                out=lg[:], in0=lg[:],
                in1=sm[:].to_broadcast([N_GRAPHS, N_ACT]), op=OP.mult)
            nc.sync.dma_start(out=out_h.ap(), in_=lg[:])

    nc.compile()
    return nc


def _stage_inputs(in_maps):
    """Start async host->device upload of the concatenated inputs."""
    import jax
    from jax.sharding import Mesh, PartitionSpec, NamedSharding
    devices = jax.devices()[:8]
    assert len(devices) >= 8
    mesh = Mesh(np.asarray(devices[:8]), ("core",))
    sh = NamedSharding(mesh, PartitionSpec("core"))
    put = {name: jax.device_put(
        np.concatenate([m[name] for m in in_maps], axis=0), sh)
        for name in in_maps[0]}
    return mesh, sh, put


def _run_fast(nc, mesh, sh, put):
    """run_bass_via_pjrt with upload already in flight (overlaps compile)."""
    import jax
    from jax.sharding import PartitionSpec
    from jax.experimental.shard_map import shard_map
    from concourse.bass2jax import (_bass_exec_p, partition_id_tensor,
                                    install_neuronx_cc_hook)
    import concourse.mybir as mybir
    install_neuronx_cc_hook()
    assert nc.dbg_addr is None
    partition_name = (nc.partition_id_tensor.name
                      if nc.partition_id_tensor else None)
    in_names, out_names, out_avals, zero_outs = [], [], [], []
    for alloc in nc.m.functions[0].allocations:
        if not isinstance(alloc, mybir.MemoryLocationSet):
            continue
        name = alloc.memorylocations[0].name
        if alloc.kind == "ExternalInput":
            if name != partition_name:
                in_names.append(name)
        elif alloc.kind == "ExternalOutput":
            out_names.append(name)
            shape = tuple(alloc.tensor_shape)
            dtype = mybir.dt.np(alloc.dtype)
            out_avals.append(jax.core.ShapedArray(shape, dtype))
            zero_outs.append(np.zeros((8 * shape[0], *shape[1:]), dtype))
    assert set(in_names) == set(put.keys()), (in_names, list(put))
    n_params = len(in_names)
    n_outs = len(out_avals)
    in_names_all = in_names + out_names + (
        [partition_name] if partition_name else [])
    donate = tuple(range(n_params, n_params + n_outs))

    def _body(*args):
        operands = list(args)
        if partition_name is not None:
            operands.append(partition_id_tensor())
        return tuple(_bass_exec_p.bind(
            *operands, out_avals=tuple(out_avals),
            in_names=tuple(in_names_all), out_names=tuple(out_names),
            lowering_input_output_aliases=(), sim_require_finite=True,
            sim_require_nnan=True, nc=nc))

    sharded = jax.jit(
        shard_map(_body, mesh=mesh,
                  in_specs=(PartitionSpec("core"),) * (n_params + n_outs),
                  out_specs=(PartitionSpec("core"),) * len(out_names),
                  check_rep=False),
        donate_argnums=donate, keep_unused=True)
    out_arrs = sharded(*[put[n] for n in in_names], *zero_outs)
    i = out_names.index("out")
    return np.asarray(out_arrs[i]).reshape(8, N_GRAPHS, N_ACT)[0]


def kernel(x, edge_index, batch, W1, b1, W2, b2, Wl, bl):
    in_maps, calls, chunks = _prep(np.asarray(x), np.asarray(edge_index),
                                   np.asarray(batch), np.asarray(W1),
                                   np.asarray(b1), np.asarray(W2),
                                   np.asarray(b2), np.asarray(Wl),
                                   np.asarray(bl))
    try:
        mesh, sh, put = _stage_inputs(in_maps)  # upload overlaps _build+jit
        nc = _build(calls, chunks)
        return np.asarray(_run_fast(nc, mesh, sh, put), dtype=np.float32)
    except Exception:
        from concourse.bass_utils import run_bass_kernel_spmd
        nc = _build(calls, chunks)
        res = run_bass_kernel_spmd(nc, in_maps, core_ids=list(range(8)))
        return np.asarray(res.results[0]["out"], dtype=np.float32)
